# revision 1
# baseline (speedup 1.0000x reference)
import sys, os
sys.path.insert(0, "/opt/trn_rl_repo")
import numpy as np
import ml_dtypes
from contextlib import ExitStack

import concourse.bass as bass
import concourse.bacc as bacc
import concourse.tile as tile
from concourse import mybir
from concourse.bass_utils import run_bass_kernel_spmd

f32 = mybir.dt.float32
bf16 = mybir.dt.bfloat16
u32 = mybir.dt.uint32
AF = mybir.ActivationFunctionType
ALU = mybir.AluOpType
AX = mybir.AxisListType
bfnp = ml_dtypes.bfloat16

B, L, D, K = 16, 4096, 1024, 5
NCORES = 8
BPC = B // NCORES          # examples per core
LC, DC = L // 128, D // 128
SCALE = 1.0 / float(np.sqrt(D))

_NC_CACHE = {}


def _build_nc():
    if "nc" in _NC_CACHE:
        return _NC_CACHE["nc"]
    nc = bacc.Bacc("TRN2", target_bir_lowering=False, debug=False,
                   num_devices=NCORES)
    dI = lambda n, s: nc.dram_tensor(n, s, bf16, kind="ExternalInput").ap()
    hthi_d = dI("hthi", [BPC, D, L])
    htlo_d = dI("htlo", [BPC, LC, 128, D])      # lc-tiled transposed-lo
    nathi_d = dI("nathi", [BPC, L, D])
    natlo_d = dI("natlo", [BPC, L, D])
    wq_d = dI("wq", [D, D]); wkt_d = dI("wkt", [D, D])
    wv_d = dI("wv", [D, D]); wct_d = dI("wct", [D, D])
    ws_d = dI("ws", [DC, 128, 3])
    i1_d = dI("i1", [1, 1])
    pid_d = nc.dram_tensor("pid", [128, 1], f32, kind="ExternalInput").ap()
    on1_d = nc.dram_tensor("on1", [1, 128], f32, kind="ExternalInput").ap()
    on128_d = nc.dram_tensor("on128", [128, 1], f32, kind="ExternalInput").ap()
    sl_d = nc.dram_tensor("sl", [BPC, L], f32, kind="ExternalOutput").ap()
    el_d = nc.dram_tensor("el", [BPC, L], f32, kind="ExternalOutput").ap()

    with tile.TileContext(nc) as tc, ExitStack() as ctx:
        res = ctx.enter_context(tc.tile_pool(name="res", bufs=1))
        stg = ctx.enter_context(tc.tile_pool(name="stg", bufs=3))
        wstg = ctx.enter_context(tc.tile_pool(name="wstg", bufs=2))
        sm = ctx.enter_context(tc.tile_pool(name="sm", bufs=2))
        big1 = ctx.enter_context(tc.tile_pool(name="big1", bufs=1))
        ps = ctx.enter_context(tc.tile_pool(name="ps", bufs=3, space="PSUM"))
        psl = ctx.enter_context(tc.tile_pool(name="psl", bufs=1, space="PSUM"))

        # resident weights: wq, wkt  [128, dci, dout]
        wq_sb = res.tile([128, DC, D], bf16)
        wkt_sb = res.tile([128, DC, D], bf16)
        for dci in range(DC):
            nc.sync.dma_start(wq_sb[:, dci, :], wq_d[dci * 128:(dci + 1) * 128, :])
            nc.sync.dma_start(wkt_sb[:, dci, :], wkt_d[dci * 128:(dci + 1) * 128, :])
        ws_sb = res.tile([128, DC, 3], bf16)
        for dc in range(DC):
            nc.sync.dma_start(ws_sb[:, dc, :], ws_d[dc])
        i1 = res.tile([1, 1], bf16); nc.sync.dma_start(i1[:], i1_d[:])
        pid = res.tile([128, 1], f32); nc.sync.dma_start(pid[:], pid_d[:])
        on1 = res.tile([1, 128], f32); nc.sync.dma_start(on1[:], on1_d[:])
        on128 = res.tile([128, 1], f32); nc.sync.dma_start(on128[:], on128_d[:])

        ht_sb = []
        for b in range(BPC):
            htt = res.tile([128, DC, L], bf16, tag=f"ht{b}")
            ht_sb.append(htt)

        for b in range(BPC):
            for dc in range(DC):
                nc.sync.dma_start(ht_sb[b][:, dc, :],
                                  hthi_d[b, dc * 128:(dc + 1) * 128, :])

            # ---- S1: start matvec (3-term) -> logits [128, 32]
            logits = sm.tile([128, LC], f32, tag="logits")
            for lc in range(LC):
                lo = stg.tile([128, D], bf16, tag="lostg")
                nc.sync.dma_start(lo[:], htlo_d[b, lc])
                pss = ps.tile([128, 3], f32, tag="pp")
                for dc in range(DC):
                    nc.tensor.matmul(pss[:, 0:2],
                                     ht_sb[b][:, dc, lc * 128:(lc + 1) * 128],
                                     ws_sb[:, dc, 0:2],
                                     start=(dc == 0), stop=(dc == DC - 1),
                                     skip_group_check=True)
                    nc.tensor.matmul(pss[:, 2:3], lo[:, dc * 128:(dc + 1) * 128],
                                     ws_sb[:, dc, 2:3],
                                     start=False, stop=(dc == DC - 1),
                                     skip_group_check=True)
                nc.vector.tensor_reduce(logits[:, lc:lc + 1], pss[:], AX.X, ALU.add)
            nc.sync.dma_start(sl_d[b:b + 1, :].rearrange("x (c p) -> (x p) c", p=128),
                              logits[:])

            # ---- S2: top-5 (exact) + softmax5 weights
            c_v = sm.tile([128, 8], f32, tag="c_v")
            c_i = sm.tile([128, 8], u32, tag="c_i")
            nc.vector.max(c_v[:], logits[:])
            nc.vector.max_index(c_i[:], c_v[:], logits[:])
            c_if = sm.tile([128, 8], f32, tag="c_if")
            nc.vector.tensor_copy(c_if[:], c_i[:])
            c_l = sm.tile([128, 8], f32, tag="c_l")
            nc.vector.tensor_scalar(c_l[:], c_if[:], 128.0, pid[:], ALU.mult, ALU.add)
            fv = big1.tile([1, 1024], f32, tag="fv")
            fl = big1.tile([1, 1024], f32, tag="fl")
            nc.sync.dma_start(fv[0:1, :], c_v[:])
            nc.sync.dma_start(fl[0:1, :], c_l[:])
            t8v = sm.tile([1, 8], f32, tag="t8v")
            t8p = sm.tile([1, 8], u32, tag="t8p")
            nc.vector.max(t8v[:], fv[:])
            nc.vector.max_index(t8p[:], t8v[:], fv[:])
            t8lf = sm.tile([1, 8], f32, tag="t8lf")
            lregs = []
            for k in range(K):
                pos = nc.values_load(t8p[0:1, k:k + 1], min_val=0, max_val=1023,
                                     skip_runtime_bounds_check=True)
                nc.vector.tensor_copy(t8lf[0:1, k:k + 1], fl[0:1, bass.ds(pos, 1)])
            t8l = sm.tile([1, 8], u32, tag="t8l")
            nc.vector.tensor_copy(t8l[0:1, 0:K], t8lf[0:1, 0:K])
            for k in range(K):
                lregs.append(nc.values_load(t8l[0:1, k:k + 1], min_val=0,
                                            max_val=L - 1,
                                            skip_runtime_bounds_check=True))
            m5 = sm.tile([1, 1], f32, tag="m5")
            nc.vector.tensor_reduce(m5[:], t8v[0:1, 0:K], AX.X, ALU.max)
            negm5 = sm.tile([1, 1], f32, tag="negm5")
            nc.vector.tensor_scalar_mul(negm5[:], m5[:], -1.0)
            e5 = sm.tile([1, K], f32, tag="e5")
            z5 = sm.tile([1, 1], f32, tag="z5")
            nc.scalar.activation(e5[:], t8v[0:1, 0:K], AF.Exp, bias=negm5[:],
                                 scale=1.0, accum_out=z5[:])
            rz5 = sm.tile([1, 1], f32, tag="rz5")
            nc.vector.reciprocal(rz5[:], z5[:])
            w5 = sm.tile([1, K], f32, tag="w5")
            nc.vector.tensor_scalar_mul(w5[:], e5[:], rz5[:])

            # ---- S3: gather S rows (hi from resident, lo via row DMA + PE transp)
            srhs = sm.tile([128, DC, K, 2], bf16, tag="srhs")
            for k in range(K):
                nc.vector.tensor_copy(srhs[:, :, k, 0:1],
                                      ht_sb[b][:, :, bass.ds(lregs[k], 1)])
                row = big1.tile([1, D], bf16, tag="row")
                nc.sync.dma_start(row[:], natlo_d[b, bass.ds(lregs[k], 1), :])
                prow = ps.tile([128, DC, 2], bf16, tag="pp")
                for dc in range(DC):
                    nc.tensor.matmul(prow[:, dc, 0:1],
                                     row[0:1, dc * 128:(dc + 1) * 128], i1[:],
                                     is_transpose=True, start=(dc == 0),
                                     stop=(dc == DC - 1), skip_group_check=True)
                nc.vector.tensor_copy(srhs[:, :, k, 1:2], prow[:, :, 0:1])

            # ---- S4: Q^T then P^T (transposed chain, split activations)
            def wstep(rhs_pair, w_sb_full, out_tag):
                outf = sm.tile([128, DC, K], f32, tag=out_tag + "f")
                for dco in range(DC):
                    psq = ps.tile([128, K, 2], f32, tag="pp")
                    for dci in range(DC):
                        nc.tensor.matmul(psq[:],
                                         w_sb_full[:, dci, dco * 128:(dco + 1) * 128],
                                         rhs_pair[:, dci, :, :],
                                         start=(dci == 0), stop=(dci == DC - 1))
                    nc.vector.tensor_reduce(outf[:, dco, :], psq[:], AX.X, ALU.add)
                pair = sm.tile([128, DC, K, 2], bf16, tag=out_tag)
                nc.vector.tensor_copy(pair[:, :, :, 0], outf[:])
                hi_f = sm.tile([128, DC, K], f32, tag=out_tag + "h")
                nc.vector.tensor_copy(hi_f[:], pair[:, :, :, 0])
                lo_f = sm.tile([128, DC, K], f32, tag=out_tag + "l")
                nc.vector.tensor_sub(lo_f[:], outf[:], hi_f[:])
                nc.vector.tensor_copy(pair[:, :, :, 1], lo_f[:])
                return pair

            qpair = wstep(srhs, wq_sb, "qp")
            ppair = wstep(qpair, wkt_sb, "pp")

            # ---- S5: scores^T + per-k softmax -> m [128, 32]
            sct = sm.tile([128, K, LC], f32, tag="sct")
            for lc in range(LC):
                pssc = ps.tile([128, K, 2], f32, tag="pp")
                for dc in range(DC):
                    nc.tensor.matmul(pssc[:],
                                     ht_sb[b][:, dc, lc * 128:(lc + 1) * 128],
                                     ppair[:, dc, :, :],
                                     start=(dc == 0), stop=(dc == DC - 1))
                nc.vector.tensor_reduce(sct[:, :, lc], pssc[:], AX.X, ALU.add)
            rmax = sm.tile([128, K], f32, tag="rmax")
            nc.vector.tensor_reduce(rmax[:], sct[:], AX.X, ALU.max)
            fm = big1.tile([1, 128 * K], f32, tag="fm")
            nc.sync.dma_start(fm[0:1, :], rmax[:])
            kmax = sm.tile([1, K], f32, tag="kmax")
            nc.vector.tensor_reduce(kmax[:],
                                    fm[0:1, :].rearrange("x (p k) -> x k p", k=K),
                                    AX.X, ALU.max)
            pbc = ps.tile([128, K], f32, tag="pp")
            nc.tensor.matmul(pbc[:], on1[:], kmax[:], start=True, stop=True)
            nbias = sm.tile([128, K], f32, tag="nbias")
            nc.vector.tensor_scalar_mul(nbias[:], pbc[:], -SCALE)
            expk = sm.tile([128, K, LC], bf16, tag="expk")
            rows = sm.tile([128, K], f32, tag="rows")
            for k in range(K):
                nc.scalar.activation(expk[:, k, :], sct[:, k, :], AF.Exp,
                                     bias=nbias[:, k:k + 1], scale=SCALE,
                                     accum_out=rows[:, k:k + 1])
            pz = ps.tile([1, K], f32, tag="pp")
            nc.tensor.matmul(pz[:], on128[:], rows[:], start=True, stop=True)
            z_sb = sm.tile([1, K], f32, tag="z_sb")
            nc.scalar.copy(z_sb[:], pz[:])
            rz = sm.tile([1, K], f32, tag="rz")
            nc.vector.reciprocal(rz[:], z_sb[:])
            c5 = sm.tile([1, K], f32, tag="c5")
            nc.vector.tensor_mul(c5[:], rz[:], w5[:])
            pcb = ps.tile([128, K], f32, tag="pp")
            nc.tensor.matmul(pcb[:], on1[:], c5[:], start=True, stop=True)
            cb = sm.tile([128, K], f32, tag="cb")
            nc.scalar.copy(cb[:], pcb[:])
            m_f = sm.tile([128, LC], f32, tag="m_f")
            nc.vector.tensor_scalar(m_f[:], expk[:, 0, :], cb[:, 0:1], None, ALU.mult)
            tmpm = sm.tile([128, LC], f32, tag="tmpm")
            for k in range(1, K):
                nc.vector.tensor_scalar(tmpm[:], expk[:, k, :], cb[:, k:k + 1],
                                        None, ALU.mult)
                nc.vector.tensor_add(m_f[:], m_f[:], tmpm[:])
            mrhs = sm.tile([128, 2, LC], bf16, tag="mrhs")
            nc.vector.tensor_copy(mrhs[:, 0, :], m_f[:])
            mh_f = sm.tile([128, LC], f32, tag="mh_f")
            nc.vector.tensor_copy(mh_f[:], mrhs[:, 0, :])
            ml_f = sm.tile([128, LC], f32, tag="ml_f")
            nc.vector.tensor_sub(ml_f[:], m_f[:], mh_f[:])
            nc.vector.tensor_copy(mrhs[:, 1, :], ml_f[:])

            # ---- S6: a_mix = H^T @ m  (stream nathi)
            psam = psl.tile([128, DC, 2], f32, tag="psam")
            for lc in range(LC):
                nat = stg.tile([128, D], bf16, tag="natstg")
                nc.sync.dma_start(nat[:], nathi_d[b, lc * 128:(lc + 1) * 128, :])
                for dc in range(DC):
                    nc.tensor.matmul(psam[:, dc, :], nat[:, dc * 128:(dc + 1) * 128],
                                     mrhs[:, :, lc],
                                     start=(lc == 0 and dc == 0),
                                     stop=(lc == LC - 1), skip_group_check=True)
            amix = sm.tile([128, DC], f32, tag="amix")
            nc.vector.tensor_reduce(amix[:], psam[:], AX.X, ALU.add)

            def split2(src_f, tag):
                pair = sm.tile([128, 2, DC], bf16, tag=tag)
                nc.vector.tensor_copy(pair[:, 0, :], src_f[:])
                h_f = sm.tile([128, DC], f32, tag=tag + "h")
                nc.vector.tensor_copy(h_f[:], pair[:, 0, :])
                l_f = sm.tile([128, DC], f32, tag=tag + "l")
                nc.vector.tensor_sub(l_f[:], src_f[:], h_f[:])
                nc.vector.tensor_copy(pair[:, 1, :], l_f[:])
                return pair

            arhs = split2(amix, "arhs")

            # ---- S7: c_mix (wv stream), g (wct stream), fold scale
            def wstep2(rhs_pair, w_d, tag):
                outf = sm.tile([128, DC], f32, tag=tag + "f")
                psc2 = psl.tile([128, DC, 2], f32, tag="psc2")
                for dci in range(DC):
                    wt = wstg.tile([128, D], bf16, tag="wstg")
                    nc.sync.dma_start(wt[:], w_d[dci * 128:(dci + 1) * 128, :])
                    for dco in range(DC):
                        nc.tensor.matmul(psc2[:, dco, :],
                                         wt[:, dco * 128:(dco + 1) * 128],
                                         rhs_pair[:, :, dci],
                                         start=(dci == 0 and dco == 0),
                                         stop=(dci == DC - 1), skip_group_check=True)
                nc.vector.tensor_reduce(outf[:], psc2[:], AX.X, ALU.add)
                return outf

            cmix = wstep2(arhs, wv_d, "cm")
            crhs = split2(cmix, "crhs")
            g_f = wstep2(crhs, wct_d, "gg")
            gs = sm.tile([128, DC], f32, tag="gs")
            nc.vector.tensor_scalar_mul(gs[:], g_f[:], SCALE)
            grhs = split2(gs, "grhs")

            # ---- S8: end matvec from resident ht
            endl = sm.tile([128, LC], f32, tag="endl")
            for lc in range(LC):
                pse = ps.tile([128, 2], f32, tag="pp")
                for dc in range(DC):
                    nc.tensor.matmul(pse[:],
                                     ht_sb[b][:, dc, lc * 128:(lc + 1) * 128],
                                     grhs[:, :, dc],
                                     start=(dc == 0), stop=(dc == DC - 1))
                nc.vector.tensor_reduce(endl[:, lc:lc + 1], pse[:], AX.X, ALU.add)
            nc.sync.dma_start(el_d[b:b + 1, :].rearrange("x (c p) -> (x p) c", p=128),
                              endl[:])

    nc.compile()
    _NC_CACHE["nc"] = nc
    return nc


def _np_reference(H, attention_mask, w_start, b_start, w_q, b_q, w_k, b_k,
                  w_v, b_v, w_cmp, b_cmp):
    NEG = -1e9
    H = H.astype(np.float32)
    pad = attention_mask == 0
    sl = (H @ w_start + b_start)[..., 0]
    sl = np.where(pad, NEG, sl)
    x = sl - sl.max(-1, keepdims=True)
    e = np.exp(x); sp = e / e.sum(-1, keepdims=True)
    idx = np.argsort(-sp, axis=-1, kind="stable")[:, :K]
    tp = np.take_along_axis(sp, idx, axis=1)
    sr = np.take_along_axis(H, idx[..., None], axis=1)
    Q = sr @ w_q + b_q
    K_ = H @ w_k + b_k
    V = H @ w_v + b_v
    sc = np.einsum('bkd,bld->bkl', Q, K_) * SCALE
    sc = np.where(pad[:, None, :], NEG, sc)
    sc = sc - sc.max(-1, keepdims=True)
    a = np.exp(sc); a = a / a.sum(-1, keepdims=True)
    ctx_ = np.einsum('bkl,bld->bkd', a, V)
    tcmp = H @ w_cmp + b_cmp
    es = np.einsum('bkd,bld->bkl', ctx_, tcmp) * SCALE
    es = np.where(pad[:, None, :], NEG, es)
    w = tp / (tp.sum(-1, keepdims=True) + 1e-9)
    el = np.einsum('bk,bkl->bl', w, es)
    el = np.where(pad, NEG, el)
    return sl, el


def kernel(**inputs):
    H = np.asarray(inputs["H"], np.float32)
    mask = np.asarray(inputs["attention_mask"])
    b_start = np.asarray(inputs["b_start"], np.float32)
    biases_zero = all(np.all(np.asarray(inputs[n]) == 0)
                      for n in ["b_q", "b_k", "b_v", "b_cmp"])
    if not bool((mask == 1).all()) or not biases_zero:
        sl, el = _np_reference(**{k: np.asarray(v) for k, v in inputs.items()})
        return np.asarray(sl, np.float32), np.asarray(el, np.float32)

    w_start = np.asarray(inputs["w_start"], np.float32)
    w_q = np.asarray(inputs["w_q"], np.float32)
    w_k = np.asarray(inputs["w_k"], np.float32)
    w_v = np.asarray(inputs["w_v"], np.float32)
    w_cmp = np.asarray(inputs["w_cmp"], np.float32)

    hi = H.astype(bfnp)
    lo = (H - hi.astype(np.float32)).astype(bfnp)
    hthi = np.ascontiguousarray(hi.transpose(0, 2, 1))               # [B, D, L]
    # htlo tiled: [B, LC, 128, D] with [b, lc, p, dc*128+j] = lo^T[b, dc*128+p ...]
    # htlo_t[b, lc, p, d] = lo[b, lc*128 + j??  -> need HT_lo[d, lc*128+j] rows d
    htlo = np.ascontiguousarray(
        lo.transpose(0, 2, 1).reshape(B, 8, 128, LC, 128)
          .transpose(0, 3, 2, 1, 4).reshape(B, LC, 128, D))
    wsh = w_start[:, 0].astype(bfnp)
    wsl = (w_start[:, 0] - wsh.astype(np.float32)).astype(bfnp)
    ws = np.stack([wsh, wsl, wsh], axis=-1).reshape(DC, 128, 3)

    nc = _build_nc()
    in_maps = []
    for c in range(NCORES):
        s = slice(c * BPC, (c + 1) * BPC)
        in_maps.append({
            "hthi": hthi[s], "htlo": htlo[s],
            "nathi": hi[s], "natlo": lo[s],
            "wq": w_q.astype(bfnp),
            "wkt": np.ascontiguousarray(w_k.T).astype(bfnp),
            "wv": w_v.astype(bfnp),
            "wct": np.ascontiguousarray(w_cmp.T).astype(bfnp),
            "ws": ws, "i1": np.ones((1, 1), bfnp),
            "pid": np.arange(128, dtype=np.float32)[:, None],
            "on1": np.ones((1, 128), np.float32),
            "on128": np.ones((128, 1), np.float32),
        })
    import time as _time
    _t0 = _time.time()
    res = run_bass_kernel_spmd(nc, in_maps, core_ids=list(range(NCORES)))
    if os.environ.get("KERNEL_TIME"):
        print(f"[kernel] device dispatch+exec wall: {_time.time() - _t0:.3f}s")
    sl = np.concatenate([r["sl"] for r in res.results], 0) + b_start[0]
    el = np.concatenate([r["el"] for r in res.results], 0)
    return sl.astype(np.float32), el.astype(np.float32)



# revision 3
# speedup vs baseline: 344.5720x; 344.5720x over previous
import sys, os
sys.path.insert(0, "/opt/trn_rl_repo")
import numpy as np
import ml_dtypes
from contextlib import ExitStack

import concourse.bass as bass
import concourse.bacc as bacc
import concourse.tile as tile
from concourse import mybir
from concourse.bass_utils import run_bass_kernel_spmd

f32 = mybir.dt.float32
bf16 = mybir.dt.bfloat16
u32 = mybir.dt.uint32
AF = mybir.ActivationFunctionType
ALU = mybir.AluOpType
AX = mybir.AxisListType
bfnp = ml_dtypes.bfloat16

B, L, D, K = 16, 4096, 1024, 5
NCORES = 8
BPC = B // NCORES          # examples per core
LC, DC = L // 128, D // 128
SCALE = 1.0 / float(np.sqrt(D))

_NC_CACHE = {}
LAST = {}


def _build_nc():
    if "nc" in _NC_CACHE:
        return _NC_CACHE["nc"]
    nc = bacc.Bacc("TRN2", target_bir_lowering=False, debug=False,
                   num_devices=NCORES)
    dI = lambda n, s: nc.dram_tensor(n, s, bf16, kind="ExternalInput").ap()
    hthi_d = dI("hthi", [BPC, D, L])
    htlo_d = dI("htlo", [BPC, LC, 128, D])      # lc-tiled transposed-lo
    nathi_d = dI("nathi", [BPC, L, D])
    natlo_d = dI("natlo", [BPC, L, D])
    wq_d = dI("wq", [D, D]); wkt_d = dI("wkt", [D, D])
    wv_d = dI("wv", [D, D]); wct_d = dI("wct", [D, D])
    ws_d = dI("ws", [DC, 128, 3])
    i1_d = dI("i1", [1, 1])
    pid_d = nc.dram_tensor("pid", [128, 1], f32, kind="ExternalInput").ap()
    on1_d = nc.dram_tensor("on1", [1, 128], f32, kind="ExternalInput").ap()
    on128_d = nc.dram_tensor("on128", [128, 1], f32, kind="ExternalInput").ap()
    sl_d = nc.dram_tensor("sl", [BPC, L], f32, kind="ExternalOutput").ap()
    el_d = nc.dram_tensor("el", [BPC, L], f32, kind="ExternalOutput").ap()

    with tile.TileContext(nc) as tc, ExitStack() as ctx:
        res = ctx.enter_context(tc.tile_pool(name="res", bufs=1))
        stg = ctx.enter_context(tc.tile_pool(name="stg", bufs=3))
        wstg = ctx.enter_context(tc.tile_pool(name="wstg", bufs=2))
        sm = ctx.enter_context(tc.tile_pool(name="sm", bufs=2))
        big1 = ctx.enter_context(tc.tile_pool(name="big1", bufs=1))
        ps = ctx.enter_context(tc.tile_pool(name="ps", bufs=3, space="PSUM"))
        psl = ctx.enter_context(tc.tile_pool(name="psl", bufs=1, space="PSUM"))

        # resident weights: wq, wkt  [128, dci, dout]
        wq_sb = res.tile([128, DC, D], bf16)
        wkt_sb = res.tile([128, DC, D], bf16)
        for dci in range(DC):
            nc.sync.dma_start(wq_sb[:, dci, :], wq_d[dci * 128:(dci + 1) * 128, :])
            nc.sync.dma_start(wkt_sb[:, dci, :], wkt_d[dci * 128:(dci + 1) * 128, :])
        ws_sb = res.tile([128, DC, 3], bf16)
        for dc in range(DC):
            nc.sync.dma_start(ws_sb[:, dc, :], ws_d[dc])
        i1 = res.tile([1, 1], bf16); nc.sync.dma_start(i1[:], i1_d[:])
        pid = res.tile([128, 1], f32); nc.sync.dma_start(pid[:], pid_d[:])
        on1 = res.tile([1, 128], f32); nc.sync.dma_start(on1[:], on1_d[:])
        on128 = res.tile([128, 1], f32); nc.sync.dma_start(on128[:], on128_d[:])

        ht_sb = []
        for b in range(BPC):
            htt = res.tile([128, DC, L], bf16, tag=f"ht{b}")
            ht_sb.append(htt)

        for b in range(BPC):
            for dc in range(DC):
                nc.sync.dma_start(ht_sb[b][:, dc, :],
                                  hthi_d[b, dc * 128:(dc + 1) * 128, :])

            # ---- S1: start matvec (3-term) -> logits [128, 32]
            logits = sm.tile([128, LC], f32, tag="logits")
            for lc in range(LC):
                lo = stg.tile([128, D], bf16, tag="lostg")
                nc.sync.dma_start(lo[:], htlo_d[b, lc])
                pss = ps.tile([128, 3], f32, tag="pp")
                for dc in range(DC):
                    nc.tensor.matmul(pss[:, 0:2],
                                     ht_sb[b][:, dc, lc * 128:(lc + 1) * 128],
                                     ws_sb[:, dc, 0:2],
                                     start=(dc == 0), stop=(dc == DC - 1),
                                     skip_group_check=True)
                    nc.tensor.matmul(pss[:, 2:3], lo[:, dc * 128:(dc + 1) * 128],
                                     ws_sb[:, dc, 2:3],
                                     start=False, stop=(dc == DC - 1),
                                     skip_group_check=True)
                nc.vector.tensor_reduce(logits[:, lc:lc + 1], pss[:], AX.X, ALU.add)
            nc.sync.dma_start(sl_d[b:b + 1, :].rearrange("x (c p) -> (x p) c", p=128),
                              logits[:])

            # ---- S2: top-5 (exact) + softmax5 weights
            c_v = sm.tile([128, 8], f32, tag="c_v")
            c_i = sm.tile([128, 8], u32, tag="c_i")
            nc.vector.max(c_v[:], logits[:])
            nc.vector.max_index(c_i[:], c_v[:], logits[:])
            c_if = sm.tile([128, 8], f32, tag="c_if")
            nc.vector.tensor_copy(c_if[:], c_i[:])
            c_l = sm.tile([128, 8], f32, tag="c_l")
            nc.vector.tensor_scalar(c_l[:], c_if[:], 128.0, pid[:], ALU.mult, ALU.add)
            fv = big1.tile([1, 1024], f32, tag="fv")
            fl = big1.tile([1, 1024], f32, tag="fl")
            nc.sync.dma_start(fv[0:1, :], c_v[:])
            nc.sync.dma_start(fl[0:1, :], c_l[:])
            t8v = sm.tile([1, 8], f32, tag="t8v")
            t8p = sm.tile([1, 8], u32, tag="t8p")
            nc.vector.max(t8v[:], fv[:])
            nc.vector.max_index(t8p[:], t8v[:], fv[:])
            t8lf = sm.tile([1, 8], f32, tag="t8lf")
            lregs = []
            for k in range(K):
                pos = nc.values_load(t8p[0:1, k:k + 1], min_val=0, max_val=1023,
                                     skip_runtime_bounds_check=True)
                nc.vector.tensor_copy(t8lf[0:1, k:k + 1], fl[0:1, bass.ds(pos, 1)])
            t8l = sm.tile([1, 8], u32, tag="t8l")
            nc.vector.tensor_copy(t8l[0:1, 0:K], t8lf[0:1, 0:K])
            for k in range(K):
                lregs.append(nc.values_load(t8l[0:1, k:k + 1], min_val=0,
                                            max_val=L - 1,
                                            skip_runtime_bounds_check=True))
            m5 = sm.tile([1, 1], f32, tag="m5")
            nc.vector.tensor_reduce(m5[:], t8v[0:1, 0:K], AX.X, ALU.max)
            negm5 = sm.tile([1, 1], f32, tag="negm5")
            nc.vector.tensor_scalar_mul(negm5[:], m5[:], -1.0)
            e5 = sm.tile([1, K], f32, tag="e5")
            z5 = sm.tile([1, 1], f32, tag="z5")
            nc.scalar.activation(e5[:], t8v[0:1, 0:K], AF.Exp, bias=negm5[:],
                                 scale=1.0, accum_out=z5[:])
            rz5 = sm.tile([1, 1], f32, tag="rz5")
            nc.vector.reciprocal(rz5[:], z5[:])
            w5 = sm.tile([1, K], f32, tag="w5")
            nc.vector.tensor_scalar_mul(w5[:], e5[:], rz5[:])

            # ---- S3: gather S rows (hi from resident, lo via row DMA + PE transp)
            srhs = sm.tile([128, DC, K, 2], bf16, tag="srhs")
            for k in range(K):
                nc.vector.tensor_copy(srhs[:, :, k, 0:1],
                                      ht_sb[b][:, :, bass.ds(lregs[k], 1)])
                row = big1.tile([1, D], bf16, tag="row")
                nc.sync.dma_start(row[:], natlo_d[b, bass.ds(lregs[k], 1), :])
                prow = ps.tile([128, DC, 2], bf16, tag="pp")
                for dc in range(DC):
                    nc.tensor.matmul(prow[:, dc, 0:1],
                                     row[0:1, dc * 128:(dc + 1) * 128], i1[:],
                                     is_transpose=True, start=(dc == 0),
                                     stop=(dc == DC - 1), skip_group_check=True)
                nc.vector.tensor_copy(srhs[:, :, k, 1:2], prow[:, :, 0:1])

            # ---- S4: Q^T then P^T (transposed chain, split activations)
            def wstep(rhs_pair, w_sb_full, out_tag):
                outf = sm.tile([128, DC, K], f32, tag=out_tag + "f")
                for dco in range(DC):
                    psq = ps.tile([128, K, 2], f32, tag="pp")
                    for dci in range(DC):
                        nc.tensor.matmul(psq[:],
                                         w_sb_full[:, dci, dco * 128:(dco + 1) * 128],
                                         rhs_pair[:, dci, :, :],
                                         start=(dci == 0), stop=(dci == DC - 1))
                    nc.vector.tensor_reduce(outf[:, dco, :], psq[:], AX.X, ALU.add)
                pair = sm.tile([128, DC, K, 2], bf16, tag=out_tag)
                nc.vector.tensor_copy(pair[:, :, :, 0], outf[:])
                hi_f = sm.tile([128, DC, K], f32, tag=out_tag + "h")
                nc.vector.tensor_copy(hi_f[:], pair[:, :, :, 0])
                lo_f = sm.tile([128, DC, K], f32, tag=out_tag + "l")
                nc.vector.tensor_sub(lo_f[:], outf[:], hi_f[:])
                nc.vector.tensor_copy(pair[:, :, :, 1], lo_f[:])
                return pair

            qpair = wstep(srhs, wq_sb, "qp")
            ppair = wstep(qpair, wkt_sb, "pp")

            # ---- S5: scores^T + per-k softmax -> m [128, 32]
            sct = sm.tile([128, K, LC], f32, tag="sct")
            for lc in range(LC):
                pssc = ps.tile([128, K, 2], f32, tag="pp")
                for dc in range(DC):
                    nc.tensor.matmul(pssc[:],
                                     ht_sb[b][:, dc, lc * 128:(lc + 1) * 128],
                                     ppair[:, dc, :, :],
                                     start=(dc == 0), stop=(dc == DC - 1))
                nc.vector.tensor_reduce(sct[:, :, lc], pssc[:], AX.X, ALU.add)
            rmax = sm.tile([128, K], f32, tag="rmax")
            nc.vector.tensor_reduce(rmax[:], sct[:], AX.X, ALU.max)
            fm = big1.tile([1, 128 * K], f32, tag="fm")
            nc.sync.dma_start(fm[0:1, :], rmax[:])
            kmax = sm.tile([1, K], f32, tag="kmax")
            nc.vector.tensor_reduce(kmax[:],
                                    fm[0:1, :].rearrange("x (p k) -> x k p", k=K),
                                    AX.X, ALU.max)
            pbc = ps.tile([128, K], f32, tag="pp")
            nc.tensor.matmul(pbc[:], on1[:], kmax[:], start=True, stop=True)
            nbias = sm.tile([128, K], f32, tag="nbias")
            nc.vector.tensor_scalar_mul(nbias[:], pbc[:], -SCALE)
            expk = sm.tile([128, K, LC], bf16, tag="expk")
            rows = sm.tile([128, K], f32, tag="rows")
            for k in range(K):
                nc.scalar.activation(expk[:, k, :], sct[:, k, :], AF.Exp,
                                     bias=nbias[:, k:k + 1], scale=SCALE,
                                     accum_out=rows[:, k:k + 1])
            pz = ps.tile([1, K], f32, tag="pp")
            nc.tensor.matmul(pz[:], on128[:], rows[:], start=True, stop=True)
            z_sb = sm.tile([1, K], f32, tag="z_sb")
            nc.scalar.copy(z_sb[:], pz[:])
            rz = sm.tile([1, K], f32, tag="rz")
            nc.vector.reciprocal(rz[:], z_sb[:])
            c5 = sm.tile([1, K], f32, tag="c5")
            nc.vector.tensor_mul(c5[:], rz[:], w5[:])
            pcb = ps.tile([128, K], f32, tag="pp")
            nc.tensor.matmul(pcb[:], on1[:], c5[:], start=True, stop=True)
            cb = sm.tile([128, K], f32, tag="cb")
            nc.scalar.copy(cb[:], pcb[:])
            m_f = sm.tile([128, LC], f32, tag="m_f")
            nc.vector.tensor_scalar(m_f[:], expk[:, 0, :], cb[:, 0:1], None, ALU.mult)
            tmpm = sm.tile([128, LC], f32, tag="tmpm")
            for k in range(1, K):
                nc.vector.tensor_scalar(tmpm[:], expk[:, k, :], cb[:, k:k + 1],
                                        None, ALU.mult)
                nc.vector.tensor_add(m_f[:], m_f[:], tmpm[:])
            mrhs = sm.tile([128, 2, LC], bf16, tag="mrhs")
            nc.vector.tensor_copy(mrhs[:, 0, :], m_f[:])
            mh_f = sm.tile([128, LC], f32, tag="mh_f")
            nc.vector.tensor_copy(mh_f[:], mrhs[:, 0, :])
            ml_f = sm.tile([128, LC], f32, tag="ml_f")
            nc.vector.tensor_sub(ml_f[:], m_f[:], mh_f[:])
            nc.vector.tensor_copy(mrhs[:, 1, :], ml_f[:])

            # ---- S6: a_mix = H^T @ m  (stream nathi)
            psam = psl.tile([128, DC, 2], f32, tag="psam")
            for lc in range(LC):
                nat = stg.tile([128, D], bf16, tag="natstg")
                nc.sync.dma_start(nat[:], nathi_d[b, lc * 128:(lc + 1) * 128, :])
                for dc in range(DC):
                    nc.tensor.matmul(psam[:, dc, :], nat[:, dc * 128:(dc + 1) * 128],
                                     mrhs[:, :, lc],
                                     start=(lc == 0 and dc == 0),
                                     stop=(lc == LC - 1), skip_group_check=True)
            amix = sm.tile([128, DC], f32, tag="amix")
            nc.vector.tensor_reduce(amix[:], psam[:], AX.X, ALU.add)

            def split2(src_f, tag):
                pair = sm.tile([128, 2, DC], bf16, tag=tag)
                nc.vector.tensor_copy(pair[:, 0, :], src_f[:])
                h_f = sm.tile([128, DC], f32, tag=tag + "h")
                nc.vector.tensor_copy(h_f[:], pair[:, 0, :])
                l_f = sm.tile([128, DC], f32, tag=tag + "l")
                nc.vector.tensor_sub(l_f[:], src_f[:], h_f[:])
                nc.vector.tensor_copy(pair[:, 1, :], l_f[:])
                return pair

            arhs = split2(amix, "arhs")

            # ---- S7: c_mix (wv stream), g (wct stream), fold scale
            def wstep2(rhs_pair, w_d, tag):
                outf = sm.tile([128, DC], f32, tag=tag + "f")
                psc2 = psl.tile([128, DC, 2], f32, tag="psc2")
                for dci in range(DC):
                    wt = wstg.tile([128, D], bf16, tag="wstg")
                    nc.sync.dma_start(wt[:], w_d[dci * 128:(dci + 1) * 128, :])
                    for dco in range(DC):
                        nc.tensor.matmul(psc2[:, dco, :],
                                         wt[:, dco * 128:(dco + 1) * 128],
                                         rhs_pair[:, :, dci],
                                         start=(dci == 0 and dco == 0),
                                         stop=(dci == DC - 1), skip_group_check=True)
                nc.vector.tensor_reduce(outf[:], psc2[:], AX.X, ALU.add)
                return outf

            cmix = wstep2(arhs, wv_d, "cm")
            crhs = split2(cmix, "crhs")
            g_f = wstep2(crhs, wct_d, "gg")
            gs = sm.tile([128, DC], f32, tag="gs")
            nc.vector.tensor_scalar_mul(gs[:], g_f[:], SCALE)
            grhs = split2(gs, "grhs")

            # ---- S8: end matvec from resident ht
            endl = sm.tile([128, LC], f32, tag="endl")
            for lc in range(LC):
                pse = ps.tile([128, 2], f32, tag="pp")
                for dc in range(DC):
                    nc.tensor.matmul(pse[:],
                                     ht_sb[b][:, dc, lc * 128:(lc + 1) * 128],
                                     grhs[:, :, dc],
                                     start=(dc == 0), stop=(dc == DC - 1))
                nc.vector.tensor_reduce(endl[:, lc:lc + 1], pse[:], AX.X, ALU.add)
            nc.sync.dma_start(el_d[b:b + 1, :].rearrange("x (c p) -> (x p) c", p=128),
                              endl[:])

    nc.compile()
    _NC_CACHE["nc"] = nc
    return nc


def _np_reference(H, attention_mask, w_start, b_start, w_q, b_q, w_k, b_k,
                  w_v, b_v, w_cmp, b_cmp):
    NEG = -1e9
    H = H.astype(np.float32)
    pad = attention_mask == 0
    sl = (H @ w_start + b_start)[..., 0]
    sl = np.where(pad, NEG, sl)
    x = sl - sl.max(-1, keepdims=True)
    e = np.exp(x); sp = e / e.sum(-1, keepdims=True)
    idx = np.argsort(-sp, axis=-1, kind="stable")[:, :K]
    tp = np.take_along_axis(sp, idx, axis=1)
    sr = np.take_along_axis(H, idx[..., None], axis=1)
    Q = sr @ w_q + b_q
    K_ = H @ w_k + b_k
    V = H @ w_v + b_v
    sc = np.einsum('bkd,bld->bkl', Q, K_) * SCALE
    sc = np.where(pad[:, None, :], NEG, sc)
    sc = sc - sc.max(-1, keepdims=True)
    a = np.exp(sc); a = a / a.sum(-1, keepdims=True)
    ctx_ = np.einsum('bkl,bld->bkd', a, V)
    tcmp = H @ w_cmp + b_cmp
    es = np.einsum('bkd,bld->bkl', ctx_, tcmp) * SCALE
    es = np.where(pad[:, None, :], NEG, es)
    w = tp / (tp.sum(-1, keepdims=True) + 1e-9)
    el = np.einsum('bk,bkl->bl', w, es)
    el = np.where(pad, NEG, el)
    return sl, el


def kernel(**inputs):
    H = np.asarray(inputs["H"], np.float32)
    mask = np.asarray(inputs["attention_mask"])
    b_start = np.asarray(inputs["b_start"], np.float32)
    biases_zero = all(np.all(np.asarray(inputs[n]) == 0)
                      for n in ["b_q", "b_k", "b_v", "b_cmp"])
    if not bool((mask == 1).all()) or not biases_zero:
        sl, el = _np_reference(**{k: np.asarray(v) for k, v in inputs.items()})
        return np.asarray(sl, np.float32), np.asarray(el, np.float32)

    w_start = np.asarray(inputs["w_start"], np.float32)
    w_q = np.asarray(inputs["w_q"], np.float32)
    w_k = np.asarray(inputs["w_k"], np.float32)
    w_v = np.asarray(inputs["w_v"], np.float32)
    w_cmp = np.asarray(inputs["w_cmp"], np.float32)

    hi = H.astype(bfnp)
    lo = (H - hi.astype(np.float32)).astype(bfnp)
    hthi = np.ascontiguousarray(hi.transpose(0, 2, 1))               # [B, D, L]
    # htlo tiled: [B, LC, 128, D] with [b, lc, p, dc*128+j] = lo^T[b, dc*128+p ...]
    # htlo_t[b, lc, p, d] = lo[b, lc*128 + j??  -> need HT_lo[d, lc*128+j] rows d
    htlo = np.ascontiguousarray(
        lo.transpose(0, 2, 1).reshape(B, 8, 128, LC, 128)
          .transpose(0, 3, 2, 1, 4).reshape(B, LC, 128, D))
    wsh = w_start[:, 0].astype(bfnp)
    wsl = (w_start[:, 0] - wsh.astype(np.float32)).astype(bfnp)
    ws = np.stack([wsh, wsl, wsh], axis=-1).reshape(DC, 128, 3)

    nc = _build_nc()
    in_maps = []
    for c in range(NCORES):
        s = slice(c * BPC, (c + 1) * BPC)
        in_maps.append({
            "hthi": hthi[s], "htlo": htlo[s],
            "nathi": hi[s], "natlo": lo[s],
            "wq": w_q.astype(bfnp),
            "wkt": np.ascontiguousarray(w_k.T).astype(bfnp),
            "wv": w_v.astype(bfnp),
            "wct": np.ascontiguousarray(w_cmp.T).astype(bfnp),
            "ws": ws, "i1": np.ones((1, 1), bfnp),
            "pid": np.arange(128, dtype=np.float32)[:, None],
            "on1": np.ones((1, 128), np.float32),
            "on128": np.ones((128, 1), np.float32),
        })
    import time as _time
    _t0 = _time.time()
    kw = {}
    if os.environ.get("KERNEL_PROFILE"):
        kw = dict(trace=True,
                  tmpdir=os.environ.get("KERNEL_PROFILE_DIR") or None,
                  trace_cores=[int(x) for x in
                               os.environ.get("KERNEL_TRACE_CORES", "0").split(",")])
    res = run_bass_kernel_spmd(nc, in_maps, core_ids=list(range(NCORES)), **kw)
    LAST["res"] = res
    if os.environ.get("KERNEL_TIME"):
        print(f"[kernel] device dispatch+exec wall: {_time.time() - _t0:.3f}s")
    sl = np.concatenate([r["sl"] for r in res.results], 0) + b_start[0]
    el = np.concatenate([r["el"] for r in res.results], 0)
    return sl.astype(np.float32), el.astype(np.float32)



# revision 22
# speedup vs baseline: 428.7097x; 1.2442x over previous
import sys, os
sys.path.insert(0, "/opt/trn_rl_repo")
import numpy as np
import ml_dtypes
from contextlib import ExitStack

import concourse.bass as bass
import concourse.bacc as bacc
import concourse.tile as tile
from concourse import mybir
from concourse.bass_utils import run_bass_kernel_spmd

f32 = mybir.dt.float32
bf16 = mybir.dt.bfloat16
u32 = mybir.dt.uint32
AF = mybir.ActivationFunctionType
ALU = mybir.AluOpType
AX = mybir.AxisListType
bfnp = ml_dtypes.bfloat16

B, L, D, K = 16, 4096, 1024, 5
NCORES = 8
BPC = B // NCORES          # examples per core
DC = D // 128              # 8 contraction chunks
NJ = L // 512              # 8 moving chunks of 512
NCAND = 8                  # top-8 candidates, exact top-5 refinement
SCALE = 1.0 / float(np.sqrt(D))

_NC_CACHE = {}
LAST = {}


def _build_nc():
    if "nc" in _NC_CACHE:
        return _NC_CACHE["nc"]
    nc = bacc.Bacc("TRN2", target_bir_lowering=False, debug=False,
                   num_devices=NCORES)
    dI = lambda n, s, dt=bf16: nc.dram_tensor(n, s, dt, kind="ExternalInput").ap()
    ht_d = dI("ht", [BPC, DC, 128, L])          # H^T hi, chunked by d
    hrow_d = dI("hrow", [BPC, L, D], f32)       # raw fp32 H for row gather
    wq_d = dI("wq", [D, D]); wkt_d = dI("wkt", [D, D])
    wv_d = dI("wv", [D, D]); wct_d = dI("wct", [D, D])
    wsb_d = dI("wsb", [DC, 128, 1])             # w_start hi, chunked
    wsf8_d = dI("wsf8", [NCAND, D], f32)        # w_start fp32, replicated rows
    i8_d = dI("i8", [NCAND, NCAND], f32)
    pid8_d = dI("pid8", [NCAND, 1], f32)        # j*512 per partition
    ones8_d = dI("ones8", [NCAND, 1])
    on128_d = dI("on128", [1, 128])
    sl_d = nc.dram_tensor("sl", [BPC, L], f32, kind="ExternalOutput").ap()
    el_d = nc.dram_tensor("el", [BPC, L], f32, kind="ExternalOutput").ap()

    with tile.TileContext(nc) as tc, ExitStack() as ctx:
        res = ctx.enter_context(tc.tile_pool(name="res", bufs=1))
        wstg = ctx.enter_context(tc.tile_pool(name="wstg", bufs=4))
        sm = ctx.enter_context(tc.tile_pool(name="sm", bufs=1))
        pbig = ctx.enter_context(tc.tile_pool(name="pbig", bufs=3, space="PSUM"))
        psm = ctx.enter_context(tc.tile_pool(name="psm", bufs=2, space="PSUM"))

        # ---- resident loads
        ht_sb = []
        for b in range(BPC):
            htt = res.tile([128, DC, L], bf16, tag=f"ht{b}", name=f"ht{b}")
            ht_sb.append(htt)
            for dc in range(DC):
                nc.sync.dma_start(htt[:, dc, :], ht_d[b, dc])
        wsb = res.tile([128, DC, 1], bf16)
        for dc in range(DC):
            nc.sync.dma_start(wsb[:, dc, :], wsb_d[dc])
        wsf8 = res.tile([NCAND, D], f32); nc.sync.dma_start(wsf8[:], wsf8_d[:])
        i8 = res.tile([NCAND, NCAND], f32); nc.sync.dma_start(i8[:], i8_d[:])
        pid8 = res.tile([NCAND, 1], f32); nc.sync.dma_start(pid8[:], pid8_d[:])
        ones8 = res.tile([NCAND, 1], bf16); nc.sync.dma_start(ones8[:], ones8_d[:])
        on128 = res.tile([1, 128], bf16); nc.sync.dma_start(on128[:], on128_d[:])

        # ---- S1: start logits, flipped orientation (ws stationary, ht moving)
        logits8 = []
        for b in range(BPC):
            l8 = sm.tile([NJ, 512], f32, tag=f"l8_{b}", name=f"l8_{b}")
            logits8.append(l8)
            for j in range(NJ):
                psL = pbig.tile([16, 512], f32, tag="mm", name="psL")
                for dc in range(DC):
                    nc.tensor.matmul(psL[0:1, :], wsb[:, dc, :],
                                     ht_sb[b][:, dc, j * 512:(j + 1) * 512],
                                     start=(dc == 0), stop=(dc == DC - 1))
                ltmp = sm.tile([1, 512], f32, tag="ltmp", bufs=2, name="ltmp")
                nc.scalar.copy(ltmp[:], psL[0:1, :])
                nc.sync.dma_start(l8[j:j + 1, :], ltmp[:])
                nc.sync.dma_start(sl_d[b:b + 1, j * 512:(j + 1) * 512], ltmp[:])

        # ---- S2: top-8 candidates + exact fp32 refinement -> masked weights,
        #      then S3: transpose gathered rows into srhs (per example)
        srhs = sm.tile([128, DC, BPC, 2, NCAND], bf16, tag="srhs", name="srhs")
        sr_hf = sm.tile([128, DC, NCAND], f32, tag="sr_hf", name="sr_hf")
        sr_lf = sm.tile([128, DC, NCAND], f32, tag="sr_lf", name="sr_lf")
        wn8_sb = []
        for b in range(BPC):
            c_v = sm.tile([NJ, 8], f32, tag="c_v", name="c_v")
            c_i = sm.tile([NJ, 8], u32, tag="c_i", name="c_i")
            nc.vector.max(c_v[:], logits8[b][:])
            nc.vector.max_index(c_i[:], c_v[:], logits8[b][:])
            c_if = sm.tile([NJ, 8], f32, tag="c_if", name="c_if")
            nc.vector.tensor_copy(c_if[:], c_i[:])
            c_l = sm.tile([NJ, 8], f32, tag="c_l", name="c_l")
            nc.vector.tensor_scalar(c_l[:], c_if[:], pid8[:], None, ALU.add)
            fv = sm.tile([1, 64], f32, tag="fv", name="fv")
            fl = sm.tile([1, 64], f32, tag="fl", name="fl")
            nc.sync.dma_start(fv[0:1, :], c_v[:])
            nc.sync.dma_start(fl[0:1, :], c_l[:])
            t8v = sm.tile([1, 8], f32, tag="t8v", name="t8v")
            t8p = sm.tile([1, 8], u32, tag="t8p", name="t8p")
            nc.vector.max(t8v[:], fv[:])
            nc.vector.max_index(t8p[:], t8v[:], fv[:])
            t8lf = sm.tile([1, 8], f32, tag="t8lf", name="t8lf")
            for k in range(NCAND):
                pos = nc.values_load(t8p[0:1, k:k + 1], min_val=0, max_val=63,
                                     skip_runtime_bounds_check=True)
                nc.vector.tensor_copy(t8lf[0:1, k:k + 1], fl[0:1, bass.ds(pos, 1)])
            t8l = sm.tile([1, 8], u32, tag="t8l", name="t8l")
            nc.vector.tensor_copy(t8l[:], t8lf[:])
            rows = sm.tile([NCAND, D], f32, tag="rows", name="rows")
            for k in range(NCAND):
                lreg = nc.values_load(t8l[0:1, k:k + 1], min_val=0, max_val=L - 1,
                                      skip_runtime_bounds_check=True)
                nc.sync.dma_start(rows[k:k + 1, :], hrow_d[b, bass.ds(lreg, 1), :])
            # exact fp32 logits for the 8 candidates (f32 products so the
            # reduce is fp32-exact; tensor_tensor_reduce is avoided — it
            # crashes the device on this runtime)
            prod = sm.tile([NCAND, D], f32, tag="scr", name="prod")
            e8 = sm.tile([NCAND, 1], f32, tag="e8", name="e8")
            nc.vector.tensor_mul(prod[:], rows[:], wsf8[:])
            nc.vector.tensor_reduce(e8[:], prod[:], AX.X, ALU.add)
            e8r = sm.tile([1, 8], f32, tag="e8r", name="e8r")
            nc.sync.dma_start(e8r[0:1, :], e8[:])
            s8 = sm.tile([1, 8], f32, tag="s8", name="s8")
            nc.vector.max(s8[:], e8r[:])
            thr = sm.tile([1, 1], f32, tag="thr", name="thr")
            nc.vector.tensor_add(thr[:], s8[0:1, K - 1:K], s8[0:1, K:K + 1])
            nc.vector.tensor_scalar_mul(thr[:], thr[:], 0.5)
            msk = sm.tile([1, 8], f32, tag="msk", name="msk")
            nc.vector.tensor_scalar(msk[:], e8r[:], thr[:], None, ALU.is_gt)
            negmx = sm.tile([1, 1], f32, tag="negmx", name="negmx")
            nc.vector.tensor_scalar_mul(negmx[:], s8[0:1, 0:1], -1.0)
            ew = sm.tile([1, 8], f32, tag="ew", name="ew")
            nc.scalar.activation(ew[:], e8r[:], AF.Exp, bias=negmx[:], scale=1.0)
            w8m = sm.tile([1, 8], f32, tag="w8m", name="w8m")
            nc.vector.tensor_mul(w8m[:], ew[:], msk[:])
            sw = sm.tile([1, 1], f32, tag="sw", name="sw")
            nc.vector.tensor_reduce(sw[:], w8m[:], AX.X, ALU.add)
            rsw = sm.tile([1, 1], f32, tag="rsw", name="rsw")
            nc.vector.reciprocal(rsw[:], sw[:])
            wn = sm.tile([1, 8], f32, tag="wn", name="wn")
            nc.vector.tensor_scalar_mul(wn[:], w8m[:], rsw[:])
            wn8 = sm.tile([NCAND, 1], f32, tag=f"wn8_{b}", name=f"wn8_{b}")
            wn8_sb.append(wn8)
            nc.sync.dma_start(wn8[:, 0:1], wn[0:1, :])

            # S3 for this example: PE transpose of the gathered fp32 rows
            psr = psm.tile([128, DC, NCAND], f32, tag="sm", name="psr")
            for dc in range(DC):
                nc.tensor.matmul(psr[:, dc, :],
                                 rows[:, dc * 128:(dc + 1) * 128], i8[:],
                                 is_transpose=True, start=True, stop=True,
                                 skip_group_check=True)
            nc.vector.tensor_copy(srhs[:, :, b, 0, :], psr[:])
            nc.vector.tensor_copy(sr_hf[:], srhs[:, :, b, 0, :])
            nc.vector.tensor_sub(sr_lf[:], psr[:], sr_hf[:])
            nc.vector.tensor_copy(srhs[:, :, b, 1, :], sr_lf[:])

        # ---- S4: Q^T then P^T chains (weights streamed, both examples)
        def wchain(w_d, rhs, tag):
            ps4 = psm.tile([128, DC, BPC, 2, NCAND], f32, tag="sm", name="ps4")
            for dci in range(DC):
                wt = wstg.tile([128, D], bf16, tag="wt", name="wt")
                nc.sync.dma_start(wt[:], w_d[dci * 128:(dci + 1) * 128, :])
                for dco in range(DC):
                    # one global start per psum tile: a later start=True would
                    # clobber sibling regions' accumulation state in the bank
                    nc.tensor.matmul(ps4[:, dco, :, :, :],
                                     wt[:, dco * 128:(dco + 1) * 128],
                                     rhs[:, dci, :, :, :],
                                     start=(dci == 0 and dco == 0),
                                     stop=(dci == DC - 1),
                                     skip_group_check=True)
            qf = sm.tile([128, DC, BPC, NCAND], f32, tag=tag + "f", name=tag + "f")
            nc.vector.tensor_copy(qf[:], ps4[:, :, :, 0, :])
            nc.vector.tensor_add(qf[:], qf[:], ps4[:, :, :, 1, :])
            pair = sm.tile([128, DC, BPC, 2, NCAND], bf16, tag=tag, name=tag)
            nc.vector.tensor_copy(pair[:, :, :, 0, :], qf[:])
            hf = sm.tile([128, DC, BPC, NCAND], f32, tag=tag + "h", name=tag + "h")
            nc.vector.tensor_copy(hf[:], pair[:, :, :, 0, :])
            lf = sm.tile([128, DC, BPC, NCAND], f32, tag=tag + "l", name=tag + "l")
            nc.vector.tensor_sub(lf[:], qf[:], hf[:])
            nc.vector.tensor_copy(pair[:, :, :, 1, :], lf[:])
            return pair

        qpair = wchain(wq_d, srhs, "qp")
        ppair = wchain(wkt_d, qpair, "pp")

        # ---- S5 both examples first (PE back-to-back), chunk maxes on the fly
        sct_sb, mxc_sb = [], []
        for b in range(BPC):
            sct = sm.tile([NCAND, L], bf16, tag=f"sct{b}", name=f"sct{b}")
            sct_sb.append(sct)
            mxc = sm.tile([NCAND, NJ], f32, tag=f"mxc{b}", name=f"mxc{b}")
            mxc_sb.append(mxc)
            for j in range(NJ):
                ps5 = pbig.tile([16, 512], f32, tag="mm", name="ps5")
                for dc in range(DC):
                    nc.tensor.matmul(ps5[:], ppair[:, dc, b, :, :],
                                     ht_sb[b][:, dc, j * 512:(j + 1) * 512],
                                     start=(dc == 0), stop=(dc == DC - 1))
                # fold hi+lo rows: engines cannot cross partition bases, so
                # stage via scalar copy + DMA partition move, then DVE add
                cp5 = sm.tile([16, 512], f32, tag="cp5", bufs=1, name="cp5")
                nc.scalar.copy(cp5[:], ps5[:])
                cp5b = sm.tile([NCAND, 512], f32, tag="cp5b", bufs=1, name="cp5b")
                nc.sync.dma_start(cp5b[:], cp5[NCAND:16, :])
                nc.vector.tensor_add(sct[:, j * 512:(j + 1) * 512],
                                     cp5[0:NCAND, :], cp5b[:])
                nc.vector.tensor_reduce(mxc[:, j:j + 1],
                                        sct[:, j * 512:(j + 1) * 512],
                                        AX.X, ALU.max)

        # ---- softmax + m broadcast + S6 per example (shared big scratch)
        amix = sm.tile([128, DC, BPC], f32, tag="amix", name="amix")
        for b in range(BPC):
            mx8 = sm.tile([NCAND, 1], f32, tag="mx8", name="mx8")
            nc.vector.tensor_reduce(mx8[:], mxc_sb[b][:], AX.X, ALU.max)
            nbias = sm.tile([NCAND, 1], f32, tag="nbias", name="nbias")
            nc.vector.tensor_scalar_mul(nbias[:], mx8[:], -SCALE)
            ek = sm.tile([NCAND, L], bf16, tag="ek", name="ek")
            z8 = sm.tile([NCAND, 1], f32, tag="z8", name="z8")
            nc.scalar.activation(ek[:], sct_sb[b][:], AF.Exp, bias=nbias[:],
                                 scale=SCALE, accum_out=z8[:])
            rz8 = sm.tile([NCAND, 1], f32, tag="rz8", name="rz8")
            nc.vector.reciprocal(rz8[:], z8[:])
            c8 = sm.tile([NCAND, 1], f32, tag="c8", name="c8")
            nc.vector.tensor_mul(c8[:], wn8_sb[b][:], rz8[:])
            nc.vector.tensor_scalar_mul(ek[:], ek[:], c8[:])   # ek *= c8
            mb128 = sm.tile([128, L], bf16, tag="mb128", name="mb128")
            for j in range(NJ):
                pm = pbig.tile([16, 512], f32, tag="mm", name="pm")
                nc.tensor.matmul(pm[0:1, :], ones8[:],
                                 ek[:, j * 512:(j + 1) * 512],
                                 start=True, stop=True)
                m1 = sm.tile([1, 512], bf16, tag="m1", bufs=2, name="m1")
                nc.scalar.copy(m1[:], pm[0:1, :])
                pmb = pbig.tile([128, 512], f32, tag="mm", name="pmb")
                nc.tensor.matmul(pmb[:], on128[:], m1[:], start=True, stop=True)
                nc.scalar.copy(mb128[:, j * 512:(j + 1) * 512], pmb[:])
            # S6: a_mix[d] = sum_l ht[d, l] * m[l]  (mult + reduce on DVE)
            scr = sm.tile([128, L], bf16, tag="scr", name="scr")
            for dc in range(DC):
                nc.vector.tensor_mul(scr[:], ht_sb[b][:, dc, :], mb128[:])
                nc.vector.tensor_reduce(amix[:, dc, b:b + 1], scr[:],
                                        AX.X, ALU.add)

        # ---- split helper [128, DC, BPC] f32 -> [128, 2, DC, BPC] bf16
        def split2(src, tag):
            pair = sm.tile([128, DC, 2, BPC], bf16, tag=tag, name=tag)
            nc.vector.tensor_copy(pair[:, :, 0, :], src[:])
            hf = sm.tile([128, DC, BPC], f32, tag=tag + "h", name=tag + "h")
            nc.vector.tensor_copy(hf[:], pair[:, :, 0, :])
            lf = sm.tile([128, DC, BPC], f32, tag=tag + "l", name=tag + "l")
            nc.vector.tensor_sub(lf[:], src[:], hf[:])
            nc.vector.tensor_copy(pair[:, :, 1, :], lf[:])
            return pair

        arhs = split2(amix, "arhs")

        # ---- S7: c_mix (wv), g (wct), shared weight streams for both examples
        def wchain2(w_d, rhs, tag):
            ps7 = psm.tile([128, DC, 2, BPC], f32, tag="sm", name="ps7")
            for dci in range(DC):
                wt = wstg.tile([128, D], bf16, tag="wt", name="wt")
                nc.sync.dma_start(wt[:], w_d[dci * 128:(dci + 1) * 128, :])
                for dco in range(DC):
                    nc.tensor.matmul(ps7[:, dco, :, :],
                                     wt[:, dco * 128:(dco + 1) * 128],
                                     rhs[:, dci, :, :],
                                     start=(dci == 0 and dco == 0),
                                     stop=(dci == DC - 1),
                                     skip_group_check=True)
            outf = sm.tile([128, DC, BPC], f32, tag=tag, name=tag)
            nc.vector.tensor_copy(outf[:], ps7[:, :, 0, :])
            nc.vector.tensor_add(outf[:], outf[:], ps7[:, :, 1, :])
            return outf

        cmix = wchain2(wv_d, arhs, "cm")
        crhs = split2(cmix, "crhs")
        g_f = wchain2(wct_d, crhs, "gg")
        gs = sm.tile([128, DC, BPC], f32, tag="gs", name="gs")
        nc.vector.tensor_scalar_mul(gs[:], g_f[:], SCALE)
        grhs = split2(gs, "grhs")

        # ---- S8: end logits from resident ht
        for b in range(BPC):
            for j in range(NJ):
                ps8 = pbig.tile([16, 512], f32, tag="mm", name="ps8")
                for dc in range(DC):
                    nc.tensor.matmul(ps8[0:2, :], grhs[:, dc, :, b],
                                     ht_sb[b][:, dc, j * 512:(j + 1) * 512],
                                     start=(dc == 0), stop=(dc == DC - 1))
                cp8 = sm.tile([2, 512], f32, tag="cp5", bufs=1, name="cp8")
                nc.scalar.copy(cp8[:], ps8[0:2, :])
                cp8b = sm.tile([1, 512], f32, tag="cp5b", bufs=1, name="cp8b")
                nc.sync.dma_start(cp8b[:], cp8[1:2, :])
                etmp = sm.tile([1, 512], f32, tag="ltmp", bufs=2, name="etmp")
                nc.vector.tensor_add(etmp[:], cp8[0:1, :], cp8b[:])
                nc.sync.dma_start(el_d[b:b + 1, j * 512:(j + 1) * 512], etmp[:])

    if os.environ.get("KERNEL_BUILD_INFO"):
        print(f"[kernel] sbuf remaining: {nc.sbuf_bytes_remaining} bytes")
    nc.compile()
    _NC_CACHE["nc"] = nc
    return nc


def _np_reference(H, attention_mask, w_start, b_start, w_q, b_q, w_k, b_k,
                  w_v, b_v, w_cmp, b_cmp):
    NEG = -1e9
    H = H.astype(np.float32)
    pad = attention_mask == 0
    sl = (H @ w_start + b_start)[..., 0]
    sl = np.where(pad, NEG, sl)
    x = sl - sl.max(-1, keepdims=True)
    e = np.exp(x); sp = e / e.sum(-1, keepdims=True)
    idx = np.argsort(-sp, axis=-1, kind="stable")[:, :K]
    tp = np.take_along_axis(sp, idx, axis=1)
    sr = np.take_along_axis(H, idx[..., None], axis=1)
    Q = sr @ w_q + b_q
    K_ = H @ w_k + b_k
    V = H @ w_v + b_v
    sc = np.einsum('bkd,bld->bkl', Q, K_) * SCALE
    sc = np.where(pad[:, None, :], NEG, sc)
    sc = sc - sc.max(-1, keepdims=True)
    a = np.exp(sc); a = a / a.sum(-1, keepdims=True)
    ctx_ = np.einsum('bkl,bld->bkd', a, V)
    tcmp = H @ w_cmp + b_cmp
    es = np.einsum('bkd,bld->bkl', ctx_, tcmp) * SCALE
    es = np.where(pad[:, None, :], NEG, es)
    w = tp / (tp.sum(-1, keepdims=True) + 1e-9)
    el = np.einsum('bk,bkl->bl', w, es)
    el = np.where(pad, NEG, el)
    return sl, el


def kernel(**inputs):
    H = np.asarray(inputs["H"], np.float32)
    mask = np.asarray(inputs["attention_mask"])
    b_start = np.asarray(inputs["b_start"], np.float32)
    biases_zero = all(np.all(np.asarray(inputs[n]) == 0)
                      for n in ["b_q", "b_k", "b_v", "b_cmp"])
    if not bool((mask == 1).all()) or not biases_zero:
        sl, el = _np_reference(**{k: np.asarray(v) for k, v in inputs.items()})
        return np.asarray(sl, np.float32), np.asarray(el, np.float32)

    w_start = np.asarray(inputs["w_start"], np.float32)
    w_q = np.asarray(inputs["w_q"], np.float32)
    w_k = np.asarray(inputs["w_k"], np.float32)
    w_v = np.asarray(inputs["w_v"], np.float32)
    w_cmp = np.asarray(inputs["w_cmp"], np.float32)

    hi = H.astype(bfnp)
    ht = np.ascontiguousarray(hi.transpose(0, 2, 1)).reshape(B, DC, 128, L)
    wsb = w_start[:, 0].astype(bfnp).reshape(DC, 128, 1)
    wsf8 = np.ascontiguousarray(
        np.broadcast_to(w_start[:, 0], (NCAND, D))).astype(np.float32)

    nc = _build_nc()
    in_maps = []
    for c in range(NCORES):
        s = slice(c * BPC, (c + 1) * BPC)
        in_maps.append({
            "ht": ht[s], "hrow": H[s],
            "wq": w_q.astype(bfnp),
            "wkt": np.ascontiguousarray(w_k.T).astype(bfnp),
            "wv": w_v.astype(bfnp),
            "wct": np.ascontiguousarray(w_cmp.T).astype(bfnp),
            "wsb": wsb, "wsf8": wsf8,
            "i8": np.eye(NCAND, dtype=np.float32),
            "pid8": (np.arange(NCAND, dtype=np.float32) * 512)[:, None],
            "ones8": np.ones((NCAND, 1), bfnp),
            "on128": np.ones((1, 128), bfnp),
        })
    import time as _time
    _t0 = _time.time()
    kw = {}
    if os.environ.get("KERNEL_PROFILE"):
        kw = dict(trace=True,
                  tmpdir=os.environ.get("KERNEL_PROFILE_DIR") or None,
                  trace_cores=[int(x) for x in
                               os.environ.get("KERNEL_TRACE_CORES", "0").split(",")])
    res = run_bass_kernel_spmd(nc, in_maps, core_ids=list(range(NCORES)), **kw)
    LAST["res"] = res
    if os.environ.get("KERNEL_TIME"):
        print(f"[kernel] device dispatch+exec wall: {_time.time() - _t0:.3f}s")
    sl = np.concatenate([r["sl"] for r in res.results], 0) + b_start[0]
    el = np.concatenate([r["el"] for r in res.results], 0)
    return sl.astype(np.float32), el.astype(np.float32)


# revision 25
# speedup vs baseline: 552.6232x; 1.2890x over previous
import sys, os
sys.path.insert(0, "/opt/trn_rl_repo")
import numpy as np
import ml_dtypes
from contextlib import ExitStack

import concourse.bass as bass
import concourse.bacc as bacc
import concourse.tile as tile
from concourse import mybir
from concourse.bass_utils import run_bass_kernel_spmd

f32 = mybir.dt.float32
bf16 = mybir.dt.bfloat16
u32 = mybir.dt.uint32
AF = mybir.ActivationFunctionType
ALU = mybir.AluOpType
AX = mybir.AxisListType
bfnp = ml_dtypes.bfloat16

B, L, D, K = 16, 4096, 1024, 5
NCORES = 8
BPC = B // NCORES          # examples per core
DC = D // 128              # 8 contraction chunks
NJ = L // 512              # 8 moving chunks of 512
NCAND = 8                  # top-8 candidates, exact top-5 refinement
SCALE = 1.0 / float(np.sqrt(D))

_NC_CACHE = {}
LAST = {}


def _build_nc():
    if "nc" in _NC_CACHE:
        return _NC_CACHE["nc"]
    nc = bacc.Bacc("TRN2", target_bir_lowering=False, debug=False,
                   num_devices=NCORES)
    dI = lambda n, s, dt=bf16: nc.dram_tensor(n, s, dt, kind="ExternalInput").ap()
    ht_d = dI("ht", [BPC, DC, 128, L])          # H^T hi, chunked by d
    nat_d = dI("nat", [BPC, L, D])              # H hi, natural layout
    hrow_d = dI("hrow", [BPC, L, D], f32)       # raw fp32 H for row gather
    wq_d = dI("wq", [D, D]); wkt_d = dI("wkt", [D, D])
    wv_d = dI("wv", [D, D]); wct_d = dI("wct", [D, D])
    wsb_d = dI("wsb", [DC, 128, 1])             # w_start hi, chunked
    wsf8_d = dI("wsf8", [NCAND, D], f32)        # w_start fp32, replicated rows
    i8_d = dI("i8", [NCAND, NCAND], f32)
    pid8_d = dI("pid8", [NCAND, 1], f32)        # j*512 per partition
    ones8_d = dI("ones8", [NCAND, 1])
    i32_d = dI("i32", [32, 32])
    sl_d = nc.dram_tensor("sl", [BPC, L], f32, kind="ExternalOutput").ap()
    el_d = nc.dram_tensor("el", [BPC, L], f32, kind="ExternalOutput").ap()

    with tile.TileContext(nc) as tc, ExitStack() as ctx:
        res = ctx.enter_context(tc.tile_pool(name="res", bufs=1))
        wstg = ctx.enter_context(tc.tile_pool(name="wstg", bufs=4))
        sm = ctx.enter_context(tc.tile_pool(name="sm", bufs=1))
        pbig = ctx.enter_context(tc.tile_pool(name="pbig", bufs=3, space="PSUM"))
        psm = ctx.enter_context(tc.tile_pool(name="psm", bufs=2, space="PSUM"))

        # ---- resident loads
        ht_sb = []
        for b in range(BPC):
            htt = res.tile([128, DC, L], bf16, tag=f"ht{b}", name=f"ht{b}")
            ht_sb.append(htt)
            for dc in range(DC):
                nc.gpsimd.dma_start(htt[:, dc, :], ht_d[b, dc])
        wsb = res.tile([128, DC, 1], bf16)
        for dc in range(DC):
            nc.sync.dma_start(wsb[:, dc, :], wsb_d[dc])
        wsf8 = res.tile([NCAND, D], f32); nc.sync.dma_start(wsf8[:], wsf8_d[:])
        i8 = res.tile([NCAND, NCAND], f32); nc.sync.dma_start(i8[:], i8_d[:])
        pid8 = res.tile([NCAND, 1], f32); nc.sync.dma_start(pid8[:], pid8_d[:])
        ones8 = res.tile([NCAND, 1], bf16); nc.sync.dma_start(ones8[:], ones8_d[:])
        i32 = res.tile([32, 32], bf16); nc.sync.dma_start(i32[:], i32_d[:])
        natp = ctx.enter_context(tc.tile_pool(name="natp", bufs=4))

        # ---- S1: start logits, flipped orientation (ws stationary, ht moving)
        logits8 = []
        for b in range(BPC):
            l8 = sm.tile([NJ, 512], f32, tag=f"l8_{b}", name=f"l8_{b}")
            logits8.append(l8)
            for j in range(NJ):
                psL = pbig.tile([16, 512], f32, tag="mm", name="psL")
                for dc in range(DC):
                    nc.tensor.matmul(psL[0:1, :], wsb[:, dc, :],
                                     ht_sb[b][:, dc, j * 512:(j + 1) * 512],
                                     start=(dc == 0), stop=(dc == DC - 1))
                ltmp = sm.tile([1, 512], f32, tag="ltmp", bufs=2, name="ltmp")
                nc.scalar.copy(ltmp[:], psL[0:1, :])
                nc.sync.dma_start(l8[j:j + 1, :], ltmp[:])
                nc.sync.dma_start(sl_d[b:b + 1, j * 512:(j + 1) * 512], ltmp[:])

        # ---- S2: top-8 candidates + exact fp32 refinement -> masked weights,
        #      then S3: transpose gathered rows into srhs (per example)
        srhs = sm.tile([128, DC, BPC, 2, NCAND], bf16, tag="srhs", name="srhs")
        sr_hf = sm.tile([128, DC, NCAND], f32, tag="sr_hf", name="sr_hf")
        sr_lf = sm.tile([128, DC, NCAND], f32, tag="sr_lf", name="sr_lf")
        wn8_sb = []
        for b in range(BPC):
            c_v = sm.tile([NJ, 8], f32, tag="c_v", name="c_v")
            c_i = sm.tile([NJ, 8], u32, tag="c_i", name="c_i")
            nc.vector.max(c_v[:], logits8[b][:])
            nc.vector.max_index(c_i[:], c_v[:], logits8[b][:])
            c_if = sm.tile([NJ, 8], f32, tag="c_if", name="c_if")
            nc.vector.tensor_copy(c_if[:], c_i[:])
            c_l = sm.tile([NJ, 8], f32, tag="c_l", name="c_l")
            nc.vector.tensor_scalar(c_l[:], c_if[:], pid8[:], None, ALU.add)
            fv = sm.tile([1, 64], f32, tag="fv", name="fv")
            fl = sm.tile([1, 64], f32, tag="fl", name="fl")
            nc.sync.dma_start(fv[0:1, :], c_v[:])
            nc.sync.dma_start(fl[0:1, :], c_l[:])
            t8v = sm.tile([1, 8], f32, tag="t8v", name="t8v")
            t8p = sm.tile([1, 8], u32, tag="t8p", name="t8p")
            nc.vector.max(t8v[:], fv[:])
            nc.vector.max_index(t8p[:], t8v[:], fv[:])
            t8lf = sm.tile([1, 8], f32, tag="t8lf", name="t8lf")
            for k in range(NCAND):
                pos = nc.values_load(t8p[0:1, k:k + 1], min_val=0, max_val=63,
                                     skip_runtime_bounds_check=True)
                nc.vector.tensor_copy(t8lf[0:1, k:k + 1], fl[0:1, bass.ds(pos, 1)])
            t8l = sm.tile([1, 8], u32, tag="t8l", name="t8l")
            nc.vector.tensor_copy(t8l[:], t8lf[:])
            rows = sm.tile([NCAND, D], f32, tag="rows", name="rows")
            for k in range(NCAND):
                lreg = nc.values_load(t8l[0:1, k:k + 1], min_val=0, max_val=L - 1,
                                      skip_runtime_bounds_check=True)
                nc.sync.dma_start(rows[k:k + 1, :], hrow_d[b, bass.ds(lreg, 1), :])
            # exact fp32 logits for the 8 candidates (f32 products so the
            # reduce is fp32-exact; tensor_tensor_reduce is avoided — it
            # crashes the device on this runtime)
            prod = sm.tile([NCAND, D], f32, tag="ek", name="prod")
            e8 = sm.tile([NCAND, 1], f32, tag="e8", name="e8")
            nc.vector.tensor_mul(prod[:], rows[:], wsf8[:])
            nc.vector.tensor_reduce(e8[:], prod[:], AX.X, ALU.add)
            e8r = sm.tile([1, 8], f32, tag="e8r", name="e8r")
            nc.sync.dma_start(e8r[0:1, :], e8[:])
            s8 = sm.tile([1, 8], f32, tag="s8", name="s8")
            nc.vector.max(s8[:], e8r[:])
            thr = sm.tile([1, 1], f32, tag="thr", name="thr")
            nc.vector.tensor_add(thr[:], s8[0:1, K - 1:K], s8[0:1, K:K + 1])
            nc.vector.tensor_scalar_mul(thr[:], thr[:], 0.5)
            msk = sm.tile([1, 8], f32, tag="msk", name="msk")
            nc.vector.tensor_scalar(msk[:], e8r[:], thr[:], None, ALU.is_gt)
            negmx = sm.tile([1, 1], f32, tag="negmx", name="negmx")
            nc.vector.tensor_scalar_mul(negmx[:], s8[0:1, 0:1], -1.0)
            ew = sm.tile([1, 8], f32, tag="ew", name="ew")
            nc.scalar.activation(ew[:], e8r[:], AF.Exp, bias=negmx[:], scale=1.0)
            w8m = sm.tile([1, 8], f32, tag="w8m", name="w8m")
            nc.vector.tensor_mul(w8m[:], ew[:], msk[:])
            sw = sm.tile([1, 1], f32, tag="sw", name="sw")
            nc.vector.tensor_reduce(sw[:], w8m[:], AX.X, ALU.add)
            rsw = sm.tile([1, 1], f32, tag="rsw", name="rsw")
            nc.vector.reciprocal(rsw[:], sw[:])
            wn = sm.tile([1, 8], f32, tag="wn", name="wn")
            nc.vector.tensor_scalar_mul(wn[:], w8m[:], rsw[:])
            wn8 = sm.tile([NCAND, 1], f32, tag=f"wn8_{b}", name=f"wn8_{b}")
            wn8_sb.append(wn8)
            nc.sync.dma_start(wn8[:, 0:1], wn[0:1, :])

            # S3 for this example: PE transpose of the gathered fp32 rows
            psr = psm.tile([128, DC, NCAND], f32, tag="sm", name="psr")
            for dc in range(DC):
                nc.tensor.matmul(psr[:, dc, :],
                                 rows[:, dc * 128:(dc + 1) * 128], i8[:],
                                 is_transpose=True, start=True, stop=True,
                                 skip_group_check=True)
            nc.vector.tensor_copy(srhs[:, :, b, 0, :], psr[:])
            nc.vector.tensor_copy(sr_hf[:], srhs[:, :, b, 0, :])
            nc.vector.tensor_sub(sr_lf[:], psr[:], sr_hf[:])
            nc.vector.tensor_copy(srhs[:, :, b, 1, :], sr_lf[:])

        # ---- S4: Q^T then P^T chains (weights streamed, both examples)
        def wchain(w_d, rhs, tag):
            ps4 = psm.tile([128, DC, BPC, 2, NCAND], f32, tag="sm", name="ps4")
            for dci in range(DC):
                wt = wstg.tile([128, D], bf16, tag="wt", name="wt")
                nc.gpsimd.dma_start(wt[:], w_d[dci * 128:(dci + 1) * 128, :])
                for dco in range(DC):
                    # one global start per psum tile: a later start=True would
                    # clobber sibling regions' accumulation state in the bank
                    nc.tensor.matmul(ps4[:, dco, :, :, :],
                                     wt[:, dco * 128:(dco + 1) * 128],
                                     rhs[:, dci, :, :, :],
                                     start=(dci == 0 and dco == 0),
                                     stop=(dci == DC - 1),
                                     skip_group_check=True)
            qf = sm.tile([128, DC, BPC, NCAND], f32, tag=tag + "f", name=tag + "f")
            nc.vector.tensor_copy(qf[:], ps4[:, :, :, 0, :])
            nc.vector.tensor_add(qf[:], qf[:], ps4[:, :, :, 1, :])
            pair = sm.tile([128, DC, BPC, 2, NCAND], bf16, tag=tag, name=tag)
            nc.vector.tensor_copy(pair[:, :, :, 0, :], qf[:])
            hf = sm.tile([128, DC, BPC, NCAND], f32, tag=tag + "h", name=tag + "h")
            nc.vector.tensor_copy(hf[:], pair[:, :, :, 0, :])
            lf = sm.tile([128, DC, BPC, NCAND], f32, tag=tag + "l", name=tag + "l")
            nc.vector.tensor_sub(lf[:], qf[:], hf[:])
            nc.vector.tensor_copy(pair[:, :, :, 1, :], lf[:])
            return pair

        qpair = wchain(wq_d, srhs, "qp")
        ppair = wchain(wkt_d, qpair, "pp")

        # ---- S5 both examples first (PE back-to-back), chunk maxes on the fly
        sct_sb, mxc_sb = [], []
        for b in range(BPC):
            sct = sm.tile([NCAND, L], bf16, tag=f"sct{b}", name=f"sct{b}")
            sct_sb.append(sct)
            mxc = sm.tile([NCAND, NJ], f32, tag=f"mxc{b}", name=f"mxc{b}")
            mxc_sb.append(mxc)
            for j in range(NJ):
                ps5 = pbig.tile([16, 512], f32, tag="mm", name="ps5")
                for dc in range(DC):
                    nc.tensor.matmul(ps5[:], ppair[:, dc, b, :, :],
                                     ht_sb[b][:, dc, j * 512:(j + 1) * 512],
                                     start=(dc == 0), stop=(dc == DC - 1))
                # fold hi+lo rows: engines cannot cross partition bases, so
                # stage via scalar copy + DMA partition move, then DVE add
                cp5 = sm.tile([16, 512], f32, tag="cp5", bufs=2, name="cp5")
                nc.scalar.copy(cp5[:], ps5[:])
                cp5b = sm.tile([NCAND, 512], f32, tag="cp5b", bufs=2, name="cp5b")
                nc.sync.dma_start(cp5b[:], cp5[NCAND:16, :])
                nc.vector.tensor_add(sct[:, j * 512:(j + 1) * 512],
                                     cp5[0:NCAND, :], cp5b[:])
                nc.vector.tensor_reduce(mxc[:, j:j + 1],
                                        sct[:, j * 512:(j + 1) * 512],
                                        AX.X, ALU.max)

        # ---- softmax + m broadcast + S6 per example (shared big scratch)
        amix = sm.tile([128, DC, BPC], f32, tag="amix", name="amix")
        for b in range(BPC):
            mx8 = sm.tile([NCAND, 1], f32, tag="mx8", name="mx8")
            nc.vector.tensor_reduce(mx8[:], mxc_sb[b][:], AX.X, ALU.max)
            nbias = sm.tile([NCAND, 1], f32, tag="nbias", name="nbias")
            nc.vector.tensor_scalar_mul(nbias[:], mx8[:], -SCALE)
            ek = sm.tile([NCAND, L], bf16, tag="ek", name="ek")
            z8 = sm.tile([NCAND, 1], f32, tag="z8", name="z8")
            nc.scalar.activation(ek[:], sct_sb[b][:], AF.Exp, bias=nbias[:],
                                 scale=SCALE, accum_out=z8[:])
            rz8 = sm.tile([NCAND, 1], f32, tag="rz8", name="rz8")
            nc.vector.reciprocal(rz8[:], z8[:])
            c8 = sm.tile([NCAND, 1], f32, tag="c8", name="c8")
            nc.vector.tensor_mul(c8[:], wn8_sb[b][:], rz8[:])
            nc.vector.tensor_scalar_mul(ek[:], ek[:], c8[:])   # ek *= c8
            # m as [32, 128] (nat-layout rows), then transpose to [128, 32]
            mt32 = sm.tile([32, 128], bf16, tag="mt32", name="mt32")
            for j in range(NJ):
                pm = pbig.tile([16, 512], f32, tag="mm", name="pm")
                nc.tensor.matmul(pm[0:1, :], ones8[:],
                                 ek[:, j * 512:(j + 1) * 512],
                                 start=True, stop=True)
                m1 = sm.tile([1, 512], bf16, tag="m1", bufs=2, name="m1")
                nc.scalar.copy(m1[:], pm[0:1, :])
                nc.sync.dma_start(mt32[4 * j:4 * j + 4, :], m1[:])
            pt = psm.tile([128, 32], bf16, tag="sm", name="pt")
            nc.tensor.matmul(pt[:], mt32[:], i32[:], is_transpose=True,
                             start=True, stop=True)
            mt = sm.tile([128, 32], bf16, tag="mt", name="mt")
            nc.vector.tensor_copy(mt[:], pt[:])
            # S6: a_mix = sum_l H[l, d] * m[l] on PE, streaming natural H
            ps6 = psm.tile([128, DC, 1], f32, tag="sm", name="ps6")
            for lc in range(L // 128):
                nat = natp.tile([128, D], bf16, tag="nat", name="nat")
                nc.gpsimd.dma_start(nat[:], nat_d[b, lc * 128:(lc + 1) * 128, :])
                for dc in range(DC):
                    nc.tensor.matmul(ps6[:, dc, :],
                                     nat[:, dc * 128:(dc + 1) * 128],
                                     mt[:, lc:lc + 1],
                                     start=(lc == 0 and dc == 0),
                                     stop=(lc == L // 128 - 1),
                                     skip_group_check=True)
            nc.vector.tensor_copy(amix[:, :, b:b + 1], ps6[:])

        # ---- split helper [128, DC, BPC] f32 -> [128, 2, DC, BPC] bf16
        def split2(src, tag):
            pair = sm.tile([128, DC, 2, BPC], bf16, tag=tag, name=tag)
            nc.vector.tensor_copy(pair[:, :, 0, :], src[:])
            hf = sm.tile([128, DC, BPC], f32, tag=tag + "h", name=tag + "h")
            nc.vector.tensor_copy(hf[:], pair[:, :, 0, :])
            lf = sm.tile([128, DC, BPC], f32, tag=tag + "l", name=tag + "l")
            nc.vector.tensor_sub(lf[:], src[:], hf[:])
            nc.vector.tensor_copy(pair[:, :, 1, :], lf[:])
            return pair

        arhs = split2(amix, "arhs")

        # ---- S7: c_mix (wv), g (wct), shared weight streams for both examples
        def wchain2(w_d, rhs, tag):
            ps7 = psm.tile([128, DC, 2, BPC], f32, tag="sm", name="ps7")
            for dci in range(DC):
                wt = wstg.tile([128, D], bf16, tag="wt", name="wt")
                nc.gpsimd.dma_start(wt[:], w_d[dci * 128:(dci + 1) * 128, :])
                for dco in range(DC):
                    nc.tensor.matmul(ps7[:, dco, :, :],
                                     wt[:, dco * 128:(dco + 1) * 128],
                                     rhs[:, dci, :, :],
                                     start=(dci == 0 and dco == 0),
                                     stop=(dci == DC - 1),
                                     skip_group_check=True)
            outf = sm.tile([128, DC, BPC], f32, tag=tag, name=tag)
            nc.vector.tensor_copy(outf[:], ps7[:, :, 0, :])
            nc.vector.tensor_add(outf[:], outf[:], ps7[:, :, 1, :])
            return outf

        cmix = wchain2(wv_d, arhs, "cm")
        crhs = split2(cmix, "crhs")
        g_f = wchain2(wct_d, crhs, "gg")
        gs = sm.tile([128, DC, BPC], f32, tag="gs", name="gs")
        nc.vector.tensor_scalar_mul(gs[:], g_f[:], SCALE)
        grhs = split2(gs, "grhs")

        # ---- S8: end logits from resident ht
        for b in range(BPC):
            for j in range(NJ):
                ps8 = pbig.tile([16, 512], f32, tag="mm", name="ps8")
                for dc in range(DC):
                    nc.tensor.matmul(ps8[0:2, :], grhs[:, dc, :, b],
                                     ht_sb[b][:, dc, j * 512:(j + 1) * 512],
                                     start=(dc == 0), stop=(dc == DC - 1))
                cp8 = sm.tile([2, 512], f32, tag="cp5", bufs=2, name="cp8")
                nc.scalar.copy(cp8[:], ps8[0:2, :])
                cp8b = sm.tile([1, 512], f32, tag="cp5b", bufs=2, name="cp8b")
                nc.sync.dma_start(cp8b[:], cp8[1:2, :])
                etmp = sm.tile([1, 512], f32, tag="ltmp", bufs=2, name="etmp")
                nc.vector.tensor_add(etmp[:], cp8[0:1, :], cp8b[:])
                nc.sync.dma_start(el_d[b:b + 1, j * 512:(j + 1) * 512], etmp[:])

    if os.environ.get("KERNEL_BUILD_INFO"):
        print(f"[kernel] sbuf remaining: {nc.sbuf_bytes_remaining} bytes")
    nc.compile()
    _NC_CACHE["nc"] = nc
    return nc


def _np_reference(H, attention_mask, w_start, b_start, w_q, b_q, w_k, b_k,
                  w_v, b_v, w_cmp, b_cmp):
    NEG = -1e9
    H = H.astype(np.float32)
    pad = attention_mask == 0
    sl = (H @ w_start + b_start)[..., 0]
    sl = np.where(pad, NEG, sl)
    x = sl - sl.max(-1, keepdims=True)
    e = np.exp(x); sp = e / e.sum(-1, keepdims=True)
    idx = np.argsort(-sp, axis=-1, kind="stable")[:, :K]
    tp = np.take_along_axis(sp, idx, axis=1)
    sr = np.take_along_axis(H, idx[..., None], axis=1)
    Q = sr @ w_q + b_q
    K_ = H @ w_k + b_k
    V = H @ w_v + b_v
    sc = np.einsum('bkd,bld->bkl', Q, K_) * SCALE
    sc = np.where(pad[:, None, :], NEG, sc)
    sc = sc - sc.max(-1, keepdims=True)
    a = np.exp(sc); a = a / a.sum(-1, keepdims=True)
    ctx_ = np.einsum('bkl,bld->bkd', a, V)
    tcmp = H @ w_cmp + b_cmp
    es = np.einsum('bkd,bld->bkl', ctx_, tcmp) * SCALE
    es = np.where(pad[:, None, :], NEG, es)
    w = tp / (tp.sum(-1, keepdims=True) + 1e-9)
    el = np.einsum('bk,bkl->bl', w, es)
    el = np.where(pad, NEG, el)
    return sl, el


def kernel(**inputs):
    H = np.asarray(inputs["H"], np.float32)
    mask = np.asarray(inputs["attention_mask"])
    b_start = np.asarray(inputs["b_start"], np.float32)
    biases_zero = all(np.all(np.asarray(inputs[n]) == 0)
                      for n in ["b_q", "b_k", "b_v", "b_cmp"])
    if not bool((mask == 1).all()) or not biases_zero:
        sl, el = _np_reference(**{k: np.asarray(v) for k, v in inputs.items()})
        return np.asarray(sl, np.float32), np.asarray(el, np.float32)

    w_start = np.asarray(inputs["w_start"], np.float32)
    w_q = np.asarray(inputs["w_q"], np.float32)
    w_k = np.asarray(inputs["w_k"], np.float32)
    w_v = np.asarray(inputs["w_v"], np.float32)
    w_cmp = np.asarray(inputs["w_cmp"], np.float32)

    hi = H.astype(bfnp)
    ht = np.ascontiguousarray(hi.transpose(0, 2, 1)).reshape(B, DC, 128, L)
    wsb = w_start[:, 0].astype(bfnp).reshape(DC, 128, 1)
    wsf8 = np.ascontiguousarray(
        np.broadcast_to(w_start[:, 0], (NCAND, D))).astype(np.float32)

    nc = _build_nc()
    in_maps = []
    for c in range(NCORES):
        s = slice(c * BPC, (c + 1) * BPC)
        in_maps.append({
            "ht": ht[s], "hrow": H[s], "nat": hi[s],
            "wq": w_q.astype(bfnp),
            "wkt": np.ascontiguousarray(w_k.T).astype(bfnp),
            "wv": w_v.astype(bfnp),
            "wct": np.ascontiguousarray(w_cmp.T).astype(bfnp),
            "wsb": wsb, "wsf8": wsf8,
            "i8": np.eye(NCAND, dtype=np.float32),
            "pid8": (np.arange(NCAND, dtype=np.float32) * 512)[:, None],
            "ones8": np.ones((NCAND, 1), bfnp),
            "i32": np.eye(32, dtype=np.float32).astype(bfnp),
        })
    import time as _time
    _t0 = _time.time()
    kw = {}
    if os.environ.get("KERNEL_PROFILE"):
        kw = dict(trace=True,
                  tmpdir=os.environ.get("KERNEL_PROFILE_DIR") or None,
                  trace_cores=[int(x) for x in
                               os.environ.get("KERNEL_TRACE_CORES", "0").split(",")])
    res = run_bass_kernel_spmd(nc, in_maps, core_ids=list(range(NCORES)), **kw)
    LAST["res"] = res
    if os.environ.get("KERNEL_TIME"):
        print(f"[kernel] device dispatch+exec wall: {_time.time() - _t0:.3f}s")
    sl = np.concatenate([r["sl"] for r in res.results], 0) + b_start[0]
    el = np.concatenate([r["el"] for r in res.results], 0)
    return sl.astype(np.float32), el.astype(np.float32)


# revision 29
# speedup vs baseline: 573.6784x; 1.0381x over previous
import sys, os
sys.path.insert(0, "/opt/trn_rl_repo")
import numpy as np
import ml_dtypes
from contextlib import ExitStack

import concourse.bass as bass
import concourse.bacc as bacc
import concourse.tile as tile
from concourse import mybir
from concourse.bass_utils import run_bass_kernel_spmd

f32 = mybir.dt.float32
bf16 = mybir.dt.bfloat16
u32 = mybir.dt.uint32
AF = mybir.ActivationFunctionType
ALU = mybir.AluOpType
AX = mybir.AxisListType
bfnp = ml_dtypes.bfloat16

B, L, D, K = 16, 4096, 1024, 5
NCORES = 8
BPC = B // NCORES          # examples per core
DC = D // 128              # 8 contraction chunks
NJ = L // 512              # 8 moving chunks of 512
NCAND = 8                  # top-8 candidates, exact top-5 refinement
SCALE = 1.0 / float(np.sqrt(D))

_NC_CACHE = {}
LAST = {}


def _build_nc():
    if "nc" in _NC_CACHE:
        return _NC_CACHE["nc"]
    nc = bacc.Bacc("TRN2", target_bir_lowering=False, debug=False,
                   num_devices=NCORES)
    dI = lambda n, s, dt=bf16: nc.dram_tensor(n, s, dt, kind="ExternalInput").ap()
    ht_d = dI("ht", [BPC, DC, 128, L])          # H^T hi, chunked by d
    nat_d = dI("nat", [BPC, L, D])              # H hi, natural layout
    hrow_d = dI("hrow", [BPC, L, D], f32)       # raw fp32 H for row gather
    wq_d = dI("wq", [D, D]); wkt_d = dI("wkt", [D, D])
    wv_d = dI("wv", [D, D]); wct_d = dI("wct", [D, D])
    wsb_d = dI("wsb", [DC, 128, 1])             # w_start hi, chunked
    wsf8_d = dI("wsf8", [NCAND, D], f32)        # w_start fp32, replicated rows
    i8_d = dI("i8", [NCAND, NCAND], f32)
    ones8_d = dI("ones8", [NCAND, 1])
    i32_d = dI("i32", [32, 32])
    sl_d = nc.dram_tensor("sl", [BPC, L], f32, kind="ExternalOutput").ap()
    el_d = nc.dram_tensor("el", [BPC, L], f32, kind="ExternalOutput").ap()

    with tile.TileContext(nc) as tc, ExitStack() as ctx:
        res = ctx.enter_context(tc.tile_pool(name="res", bufs=1))
        wstg = ctx.enter_context(tc.tile_pool(name="wstg", bufs=4))
        sm = ctx.enter_context(tc.tile_pool(name="sm", bufs=1))
        pbig = ctx.enter_context(tc.tile_pool(name="pbig", bufs=5, space="PSUM"))
        psm = ctx.enter_context(tc.tile_pool(name="psm", bufs=2, space="PSUM"))

        # ---- resident loads
        ht_sb = []
        for b in range(BPC):
            htt = res.tile([128, DC, L], bf16, tag=f"ht{b}", name=f"ht{b}")
            ht_sb.append(htt)
            for dc in range(DC):
                nc.gpsimd.dma_start(htt[:, dc, :], ht_d[b, dc])
        wsb = res.tile([128, DC, 1], bf16)
        for dc in range(DC):
            nc.sync.dma_start(wsb[:, dc, :], wsb_d[dc])
        wsf8 = res.tile([NCAND, D], f32); nc.sync.dma_start(wsf8[:], wsf8_d[:])
        i8 = res.tile([NCAND, NCAND], f32); nc.sync.dma_start(i8[:], i8_d[:])
        ones8 = res.tile([NCAND, 1], bf16); nc.sync.dma_start(ones8[:], ones8_d[:])
        i32 = res.tile([32, 32], bf16); nc.sync.dma_start(i32[:], i32_d[:])
        natp = ctx.enter_context(tc.tile_pool(name="natp", bufs=3))

        # ---- S1: start logits, flipped orientation (ws stationary, ht moving)
        fL_sb = []
        for b in range(BPC):
            fL = sm.tile([1, L], bf16, tag="fL", name="fL")
            fL_sb.append(fL)
            for j in range(NJ):
                psL = pbig.tile([16, 512], f32, tag="mm", name="psL")
                for dc in range(DC):
                    nc.tensor.matmul(psL[0:1, :], wsb[:, dc, :],
                                     ht_sb[b][:, dc, j * 512:(j + 1) * 512],
                                     start=(dc == 0), stop=(dc == DC - 1))
                ltmp = sm.tile([1, 512], f32, tag="ltmp", bufs=1, name="ltmp")
                nc.scalar.copy(ltmp[:], psL[0:1, :])
                nc.scalar.copy(fL[0:1, j * 512:(j + 1) * 512], psL[0:1, :])
                nc.sync.dma_start(sl_d[b:b + 1, j * 512:(j + 1) * 512], ltmp[:])

        # ---- S2: top-8 candidates + exact fp32 refinement -> masked weights,
        #      then S3: transpose gathered rows into srhs (per example)
        srhs = sm.tile([128, DC, BPC, 2, NCAND], bf16, tag="srhs", name="srhs")
        sr_hf = sm.tile([128, DC, NCAND], f32, tag="sr_hf", name="sr_hf")
        sr_lf = sm.tile([128, DC, NCAND], f32, tag="sr_lf", name="sr_lf")
        hrow_flat = hrow_d.rearrange("b l d -> (b l) d")
        wn8_sb = []
        for b in range(BPC):
            t8v = sm.tile([1, 8], bf16, tag="t8v", name="t8v")
            t8p = sm.tile([1, 8], u32, tag="t8p", name="t8p")
            nc.vector.max(t8v[:], fL_sb[b][:])
            nc.vector.max_index(t8p[:], t8v[:], fL_sb[b][:])
            t8pf = sm.tile([1, 8], f32, tag="t8pf", name="t8pf")
            nc.vector.tensor_copy(t8pf[:], t8p[:])
            nc.vector.tensor_scalar_add(t8pf[:], t8pf[:], float(b * L))
            t8pi = sm.tile([1, 8], u32, tag="t8pi", name="t8pi")
            nc.vector.tensor_copy(t8pi[:], t8pf[:])
            idx8 = sm.tile([NCAND, 1], u32, tag="idx8", name="idx8")
            nc.sync.dma_start(idx8[:, 0:1], t8pi[0:1, :])
            rows = sm.tile([NCAND, D], f32, tag="rows", name="rows")
            nc.gpsimd.indirect_dma_start(
                out=rows[:], out_offset=None, in_=hrow_flat,
                in_offset=bass.IndirectOffsetOnAxis(ap=idx8[:, 0:1], axis=0))
            # exact fp32 logits for the 8 candidates (f32 products so the
            # reduce is fp32-exact; tensor_tensor_reduce is avoided — it
            # crashes the device on this runtime)
            prod = sm.tile([NCAND, D], f32, tag="ek", name="prod")
            e8 = sm.tile([NCAND, 1], f32, tag="e8", name="e8")
            nc.vector.tensor_mul(prod[:], rows[:], wsf8[:])
            nc.vector.tensor_reduce(e8[:], prod[:], AX.X, ALU.add)
            e8r = sm.tile([1, 8], f32, tag="e8r", name="e8r")
            nc.sync.dma_start(e8r[0:1, :], e8[:])
            s8 = sm.tile([1, 8], f32, tag="s8", name="s8")
            nc.vector.max(s8[:], e8r[:])
            thr = sm.tile([1, 1], f32, tag="thr", name="thr")
            nc.vector.tensor_add(thr[:], s8[0:1, K - 1:K], s8[0:1, K:K + 1])
            nc.vector.tensor_scalar_mul(thr[:], thr[:], 0.5)
            msk = sm.tile([1, 8], f32, tag="msk", name="msk")
            nc.vector.tensor_scalar(msk[:], e8r[:], thr[:], None, ALU.is_gt)
            negmx = sm.tile([1, 1], f32, tag="negmx", name="negmx")
            nc.vector.tensor_scalar_mul(negmx[:], s8[0:1, 0:1], -1.0)
            ew = sm.tile([1, 8], f32, tag="ew", name="ew")
            nc.scalar.activation(ew[:], e8r[:], AF.Exp, bias=negmx[:], scale=1.0)
            w8m = sm.tile([1, 8], f32, tag="w8m", name="w8m")
            nc.vector.tensor_mul(w8m[:], ew[:], msk[:])
            sw = sm.tile([1, 1], f32, tag="sw", name="sw")
            nc.vector.tensor_reduce(sw[:], w8m[:], AX.X, ALU.add)
            rsw = sm.tile([1, 1], f32, tag="rsw", name="rsw")
            nc.vector.reciprocal(rsw[:], sw[:])
            wn = sm.tile([1, 8], f32, tag="wn", name="wn")
            nc.vector.tensor_scalar_mul(wn[:], w8m[:], rsw[:])
            wn8 = sm.tile([NCAND, 1], f32, tag=f"wn8_{b}", name=f"wn8_{b}")
            wn8_sb.append(wn8)
            nc.sync.dma_start(wn8[:, 0:1], wn[0:1, :])

            # S3 for this example: PE transpose of the gathered fp32 rows
            psr = psm.tile([128, DC, NCAND], f32, tag="sm", name="psr")
            for dc in range(DC):
                nc.tensor.matmul(psr[:, dc, :],
                                 rows[:, dc * 128:(dc + 1) * 128], i8[:],
                                 is_transpose=True, start=True, stop=True,
                                 skip_group_check=True)
            nc.vector.tensor_copy(srhs[:, :, b, 0, :], psr[:])
            nc.vector.tensor_copy(sr_hf[:], srhs[:, :, b, 0, :])
            nc.vector.tensor_sub(sr_lf[:], psr[:], sr_hf[:])
            nc.vector.tensor_copy(srhs[:, :, b, 1, :], sr_lf[:])

        # ---- S4: Q^T then P^T chains (weights streamed, both examples)
        def wchain(w_d, rhs, tag):
            ps4 = psm.tile([128, DC, BPC, 2, NCAND], f32, tag="sm", name="ps4")
            for dci in range(DC):
                wt = wstg.tile([128, D], bf16, tag="wt", name="wt")
                nc.gpsimd.dma_start(wt[:], w_d[dci * 128:(dci + 1) * 128, :])
                for dco in range(DC):
                    # one global start per psum tile: a later start=True would
                    # clobber sibling regions' accumulation state in the bank
                    nc.tensor.matmul(ps4[:, dco, :, :, :],
                                     wt[:, dco * 128:(dco + 1) * 128],
                                     rhs[:, dci, :, :, :],
                                     start=(dci == 0 and dco == 0),
                                     stop=(dci == DC - 1),
                                     skip_group_check=True)
            qf = sm.tile([128, DC, BPC, NCAND], f32, tag=tag + "f", name=tag + "f")
            nc.vector.tensor_copy(qf[:], ps4[:, :, :, 0, :])
            nc.vector.tensor_add(qf[:], qf[:], ps4[:, :, :, 1, :])
            pair = sm.tile([128, DC, BPC, 2, NCAND], bf16, tag=tag, name=tag)
            nc.vector.tensor_copy(pair[:, :, :, 0, :], qf[:])
            hf = sm.tile([128, DC, BPC, NCAND], f32, tag=tag + "h", name=tag + "h")
            nc.vector.tensor_copy(hf[:], pair[:, :, :, 0, :])
            nc.vector.tensor_sub(qf[:], qf[:], hf[:])
            nc.vector.tensor_copy(pair[:, :, :, 1, :], qf[:])
            return pair

        qpair = wchain(wq_d, srhs, "qp")
        ppair = wchain(wkt_d, qpair, "pp")

        # ---- S5 both examples first (PE back-to-back), chunk maxes on the fly
        sct_sb, mxc_sb = [], []
        for b in range(BPC):
            sct = sm.tile([NCAND, L], bf16, tag=f"sct{b}", name=f"sct{b}")
            sct_sb.append(sct)
            mxc = sm.tile([NCAND, NJ], f32, tag=f"mxc{b}", name=f"mxc{b}")
            mxc_sb.append(mxc)
            for j in range(NJ):
                ps5 = pbig.tile([16, 512], f32, tag="mm", name="ps5")
                for dc in range(DC):
                    nc.tensor.matmul(ps5[:], ppair[:, dc, b, :, :],
                                     ht_sb[b][:, dc, j * 512:(j + 1) * 512],
                                     start=(dc == 0), stop=(dc == DC - 1))
                # fold hi+lo rows: engines cannot cross partition bases, so
                # stage via scalar copy + DMA partition move, then DVE add
                cp5 = sm.tile([16, 512], f32, tag="cp5", bufs=2, name="cp5")
                nc.scalar.copy(cp5[:], ps5[:])
                cp5b = sm.tile([NCAND, 512], f32, tag="cp5b", bufs=2, name="cp5b")
                nc.sync.dma_start(cp5b[:], cp5[NCAND:16, :])
                nc.vector.tensor_add(sct[:, j * 512:(j + 1) * 512],
                                     cp5[0:NCAND, :], cp5b[:])
                nc.vector.tensor_reduce(mxc[:, j:j + 1],
                                        sct[:, j * 512:(j + 1) * 512],
                                        AX.X, ALU.max)

        # ---- softmax + m broadcast + S6 per example (shared big scratch)
        amix = sm.tile([128, DC, BPC], f32, tag="amix", name="amix")
        for b in range(BPC):
            mx8 = sm.tile([NCAND, 1], f32, tag="mx8", name="mx8")
            nc.vector.tensor_reduce(mx8[:], mxc_sb[b][:], AX.X, ALU.max)
            nbias = sm.tile([NCAND, 1], f32, tag="nbias", name="nbias")
            nc.vector.tensor_scalar_mul(nbias[:], mx8[:], -SCALE)
            ek = sm.tile([NCAND, L], bf16, tag="ek", name="ek")
            z8 = sm.tile([NCAND, 1], f32, tag="z8", name="z8")
            nc.scalar.activation(ek[:], sct_sb[b][:], AF.Exp, bias=nbias[:],
                                 scale=SCALE, accum_out=z8[:])
            rz8 = sm.tile([NCAND, 1], f32, tag="rz8", name="rz8")
            nc.vector.reciprocal(rz8[:], z8[:])
            c8 = sm.tile([NCAND, 1], f32, tag="c8", name="c8")
            nc.vector.tensor_mul(c8[:], wn8_sb[b][:], rz8[:])
            nc.vector.tensor_scalar_mul(ek[:], ek[:], c8[:])   # ek *= c8
            # m as [32, 128] (nat-layout rows), then transpose to [128, 32]
            mt32 = sm.tile([32, 128], bf16, tag="mt32", name="mt32")
            for j in range(NJ):
                pm = pbig.tile([16, 512], f32, tag="mm", name="pm")
                nc.tensor.matmul(pm[0:1, :], ones8[:],
                                 ek[:, j * 512:(j + 1) * 512],
                                 start=True, stop=True)
                m1 = sm.tile([1, 512], bf16, tag="m1", bufs=2, name="m1")
                nc.scalar.copy(m1[:], pm[0:1, :])
                nc.sync.dma_start(mt32[4 * j:4 * j + 4, :], m1[:])
            pt = psm.tile([128, 32], bf16, tag="sm", name="pt")
            nc.tensor.matmul(pt[:], mt32[:], i32[:], is_transpose=True,
                             start=True, stop=True)
            mt = sm.tile([128, 32], bf16, tag="mt", name="mt")
            nc.vector.tensor_copy(mt[:], pt[:])
            # S6: a_mix = sum_l H[l, d] * m[l] on PE, streaming natural H
            ps6 = psm.tile([128, DC, 1], f32, tag="sm", name="ps6")
            NLC = L // 128
            for lc2 in range(NLC // 2):
                nat = natp.tile([128, 2, D], bf16, tag="nat", name="nat")
                nc.gpsimd.dma_start(
                    nat[:],
                    nat_d[b, lc2 * 256:(lc2 + 1) * 256, :]
                    .rearrange("(i p) d -> p i d", p=128))
                for i in range(2):
                    lc = lc2 * 2 + i
                    for dc in range(DC):
                        nc.tensor.matmul(ps6[:, dc, :],
                                         nat[:, i, dc * 128:(dc + 1) * 128],
                                         mt[:, lc:lc + 1],
                                         start=(lc == 0 and dc == 0),
                                         stop=(lc == NLC - 1),
                                         skip_group_check=True)
            nc.vector.tensor_copy(amix[:, :, b:b + 1], ps6[:])

        # ---- split helper [128, DC, BPC] f32 -> [128, 2, DC, BPC] bf16
        def split2(src, tag):
            pair = sm.tile([128, DC, 2, BPC], bf16, tag=tag, name=tag)
            nc.vector.tensor_copy(pair[:, :, 0, :], src[:])
            hf = sm.tile([128, DC, BPC], f32, tag=tag + "h", name=tag + "h")
            nc.vector.tensor_copy(hf[:], pair[:, :, 0, :])
            nc.vector.tensor_sub(hf[:], src[:], hf[:])
            nc.vector.tensor_copy(pair[:, :, 1, :], hf[:])
            return pair

        arhs = split2(amix, "arhs")

        # ---- S7: c_mix (wv), g (wct), shared weight streams for both examples
        def wchain2(w_d, rhs, tag):
            ps7 = psm.tile([128, DC, 2, BPC], f32, tag="sm", name="ps7")
            for dci in range(DC):
                wt = wstg.tile([128, D], bf16, tag="wt", name="wt")
                nc.gpsimd.dma_start(wt[:], w_d[dci * 128:(dci + 1) * 128, :])
                for dco in range(DC):
                    nc.tensor.matmul(ps7[:, dco, :, :],
                                     wt[:, dco * 128:(dco + 1) * 128],
                                     rhs[:, dci, :, :],
                                     start=(dci == 0 and dco == 0),
                                     stop=(dci == DC - 1),
                                     skip_group_check=True)
            outf = sm.tile([128, DC, BPC], f32, tag=tag, name=tag)
            nc.vector.tensor_copy(outf[:], ps7[:, :, 0, :])
            nc.vector.tensor_add(outf[:], outf[:], ps7[:, :, 1, :])
            return outf

        cmix = wchain2(wv_d, arhs, "cm")
        crhs = split2(cmix, "crhs")
        g_f = wchain2(wct_d, crhs, "gg")
        gs = sm.tile([128, DC, BPC], f32, tag="gs", name="gs")
        nc.vector.tensor_scalar_mul(gs[:], g_f[:], SCALE)
        grhs = split2(gs, "grhs")

        # ---- S8: end logits from resident ht
        for b in range(BPC):
            for j in range(NJ):
                ps8 = pbig.tile([16, 512], f32, tag="mm", name="ps8")
                for dc in range(DC):
                    nc.tensor.matmul(ps8[0:2, :], grhs[:, dc, :, b],
                                     ht_sb[b][:, dc, j * 512:(j + 1) * 512],
                                     start=(dc == 0), stop=(dc == DC - 1))
                cp8 = sm.tile([2, 512], f32, tag="cp5", bufs=2, name="cp8")
                nc.scalar.copy(cp8[:], ps8[0:2, :])
                cp8b = sm.tile([1, 512], f32, tag="cp5b", bufs=2, name="cp8b")
                nc.sync.dma_start(cp8b[:], cp8[1:2, :])
                etmp = sm.tile([1, 512], f32, tag="ltmp", bufs=1, name="etmp")
                nc.vector.tensor_add(etmp[:], cp8[0:1, :], cp8b[:])
                nc.sync.dma_start(el_d[b:b + 1, j * 512:(j + 1) * 512], etmp[:])

    if os.environ.get("KERNEL_BUILD_INFO"):
        print(f"[kernel] sbuf remaining: {nc.sbuf_bytes_remaining} bytes")
    nc.compile()
    _NC_CACHE["nc"] = nc
    return nc


def _np_reference(H, attention_mask, w_start, b_start, w_q, b_q, w_k, b_k,
                  w_v, b_v, w_cmp, b_cmp):
    NEG = -1e9
    H = H.astype(np.float32)
    pad = attention_mask == 0
    sl = (H @ w_start + b_start)[..., 0]
    sl = np.where(pad, NEG, sl)
    x = sl - sl.max(-1, keepdims=True)
    e = np.exp(x); sp = e / e.sum(-1, keepdims=True)
    idx = np.argsort(-sp, axis=-1, kind="stable")[:, :K]
    tp = np.take_along_axis(sp, idx, axis=1)
    sr = np.take_along_axis(H, idx[..., None], axis=1)
    Q = sr @ w_q + b_q
    K_ = H @ w_k + b_k
    V = H @ w_v + b_v
    sc = np.einsum('bkd,bld->bkl', Q, K_) * SCALE
    sc = np.where(pad[:, None, :], NEG, sc)
    sc = sc - sc.max(-1, keepdims=True)
    a = np.exp(sc); a = a / a.sum(-1, keepdims=True)
    ctx_ = np.einsum('bkl,bld->bkd', a, V)
    tcmp = H @ w_cmp + b_cmp
    es = np.einsum('bkd,bld->bkl', ctx_, tcmp) * SCALE
    es = np.where(pad[:, None, :], NEG, es)
    w = tp / (tp.sum(-1, keepdims=True) + 1e-9)
    el = np.einsum('bk,bkl->bl', w, es)
    el = np.where(pad, NEG, el)
    return sl, el


def kernel(**inputs):
    H = np.asarray(inputs["H"], np.float32)
    mask = np.asarray(inputs["attention_mask"])
    b_start = np.asarray(inputs["b_start"], np.float32)
    biases_zero = all(np.all(np.asarray(inputs[n]) == 0)
                      for n in ["b_q", "b_k", "b_v", "b_cmp"])
    if not bool((mask == 1).all()) or not biases_zero:
        sl, el = _np_reference(**{k: np.asarray(v) for k, v in inputs.items()})
        return np.asarray(sl, np.float32), np.asarray(el, np.float32)

    w_start = np.asarray(inputs["w_start"], np.float32)
    w_q = np.asarray(inputs["w_q"], np.float32)
    w_k = np.asarray(inputs["w_k"], np.float32)
    w_v = np.asarray(inputs["w_v"], np.float32)
    w_cmp = np.asarray(inputs["w_cmp"], np.float32)

    hi = H.astype(bfnp)
    ht = np.ascontiguousarray(hi.transpose(0, 2, 1)).reshape(B, DC, 128, L)
    wsb = w_start[:, 0].astype(bfnp).reshape(DC, 128, 1)
    wsf8 = np.ascontiguousarray(
        np.broadcast_to(w_start[:, 0], (NCAND, D))).astype(np.float32)

    nc = _build_nc()
    in_maps = []
    for c in range(NCORES):
        s = slice(c * BPC, (c + 1) * BPC)
        in_maps.append({
            "ht": ht[s], "hrow": H[s], "nat": hi[s],
            "wq": w_q.astype(bfnp),
            "wkt": np.ascontiguousarray(w_k.T).astype(bfnp),
            "wv": w_v.astype(bfnp),
            "wct": np.ascontiguousarray(w_cmp.T).astype(bfnp),
            "wsb": wsb, "wsf8": wsf8,
            "i8": np.eye(NCAND, dtype=np.float32),
            "ones8": np.ones((NCAND, 1), bfnp),
            "i32": np.eye(32, dtype=np.float32).astype(bfnp),
        })
    import time as _time
    _t0 = _time.time()
    kw = {}
    if os.environ.get("KERNEL_PROFILE"):
        kw = dict(trace=True,
                  tmpdir=os.environ.get("KERNEL_PROFILE_DIR") or None,
                  trace_cores=[int(x) for x in
                               os.environ.get("KERNEL_TRACE_CORES", "0").split(",")])
    res = run_bass_kernel_spmd(nc, in_maps, core_ids=list(range(NCORES)), **kw)
    LAST["res"] = res
    if os.environ.get("KERNEL_TIME"):
        print(f"[kernel] device dispatch+exec wall: {_time.time() - _t0:.3f}s")
    sl = np.concatenate([r["sl"] for r in res.results], 0) + b_start[0]
    el = np.concatenate([r["el"] for r in res.results], 0)
    return sl.astype(np.float32), el.astype(np.float32)


# revision 31
# speedup vs baseline: 645.8986x; 1.1259x over previous
import sys, os
sys.path.insert(0, "/opt/trn_rl_repo")
import numpy as np
import ml_dtypes
from contextlib import ExitStack

import concourse.bass as bass
import concourse.bacc as bacc
import concourse.tile as tile
from concourse import mybir
from concourse.bass_utils import run_bass_kernel_spmd

f32 = mybir.dt.float32
bf16 = mybir.dt.bfloat16
u32 = mybir.dt.uint32
AF = mybir.ActivationFunctionType
ALU = mybir.AluOpType
AX = mybir.AxisListType
bfnp = ml_dtypes.bfloat16

B, L, D, K = 16, 4096, 1024, 5
NCORES = 8
BPC = B // NCORES          # examples per core
DC = D // 128              # 8 contraction chunks
NJ = L // 512              # 8 moving chunks of 512
NCAND = 8                  # top-8 candidates, exact top-5 refinement
SCALE = 1.0 / float(np.sqrt(D))

_NC_CACHE = {}
LAST = {}


def _build_nc():
    if "nc" in _NC_CACHE:
        return _NC_CACHE["nc"]
    nc = bacc.Bacc("TRN2", target_bir_lowering=False, debug=False,
                   num_devices=NCORES)
    dI = lambda n, s, dt=bf16: nc.dram_tensor(n, s, dt, kind="ExternalInput").ap()
    ht_d = dI("ht", [BPC, DC, 128, L])          # H^T hi, chunked by d
    nat_d = dI("nat", [BPC, L, D])              # H hi, natural layout
    hrow_d = dI("hrow", [BPC, L, D], f32)       # raw fp32 H for row gather
    wq_d = dI("wq", [D, D]); wkt_d = dI("wkt", [D, D])
    wv_d = dI("wv", [D, D]); wct_d = dI("wct", [D, D])
    wsb_d = dI("wsb", [DC, 128, 1])             # w_start hi, chunked
    wsf8_d = dI("wsf8", [NCAND, D], f32)        # w_start fp32, replicated rows
    i8_d = dI("i8", [NCAND, NCAND], f32)
    ones8_d = dI("ones8", [NCAND, 1])
    i32_d = dI("i32", [32, 32])
    sl_d = nc.dram_tensor("sl", [BPC, L], f32, kind="ExternalOutput").ap()
    el_d = nc.dram_tensor("el", [BPC, L], f32, kind="ExternalOutput").ap()

    with tile.TileContext(nc) as tc, ExitStack() as ctx:
        res = ctx.enter_context(tc.tile_pool(name="res", bufs=1))
        wstg = ctx.enter_context(tc.tile_pool(name="wstg", bufs=4))
        sm = ctx.enter_context(tc.tile_pool(name="sm", bufs=1))
        pbig = ctx.enter_context(tc.tile_pool(name="pbig", bufs=5, space="PSUM"))
        psm = ctx.enter_context(tc.tile_pool(name="psm", bufs=2, space="PSUM"))

        # ---- resident loads
        ht_sb = []
        for b in range(BPC):
            htt = res.tile([128, DC, L], bf16, tag=f"ht{b}", name=f"ht{b}")
            ht_sb.append(htt)
            for dc in range(DC):
                nc.gpsimd.dma_start(htt[:, dc, :], ht_d[b, dc])
        wsb = res.tile([128, DC, 1], bf16)
        for dc in range(DC):
            nc.sync.dma_start(wsb[:, dc, :], wsb_d[dc])
        wsf8 = res.tile([NCAND, D], f32); nc.sync.dma_start(wsf8[:], wsf8_d[:])
        i8 = res.tile([NCAND, NCAND], f32); nc.sync.dma_start(i8[:], i8_d[:])
        ones8 = res.tile([NCAND, 1], bf16); nc.sync.dma_start(ones8[:], ones8_d[:])
        i32 = res.tile([32, 32], bf16); nc.sync.dma_start(i32[:], i32_d[:])
        natp = ctx.enter_context(tc.tile_pool(name="natp", bufs=4))

        # ---- S1: start logits, flipped orientation (ws stationary, ht moving)
        fL_sb = []
        for b in range(BPC):
            fL = sm.tile([1, L], bf16, tag=f"sct{b}", name="fL")
            fL_sb.append(fL)
            for j in range(NJ):
                psL = pbig.tile([16, 512], f32, tag="mm", name="psL")
                for dc in range(DC):
                    nc.tensor.matmul(psL[0:1, :], wsb[:, dc, :],
                                     ht_sb[b][:, dc, j * 512:(j + 1) * 512],
                                     start=(dc == 0), stop=(dc == DC - 1))
                ltmp = sm.tile([1, 512], f32, tag="ltmp", bufs=2, name="ltmp")
                nc.scalar.copy(ltmp[:], psL[0:1, :])
                nc.scalar.copy(fL[0:1, j * 512:(j + 1) * 512], psL[0:1, :])
                nc.sync.dma_start(sl_d[b:b + 1, j * 512:(j + 1) * 512], ltmp[:])

        # ---- S2: top-8 candidates + exact fp32 refinement -> masked weights,
        #      then S3: transpose gathered rows into srhs (per example)
        srhs = sm.tile([128, DC, BPC, 2, NCAND], bf16, tag="srhs", name="srhs")
        sr_hf = sm.tile([128, DC, NCAND], f32, tag="sr_hf", name="sr_hf")
        sr_lf = sm.tile([128, DC, NCAND], f32, tag="sr_lf", name="sr_lf")
        hrow_flat = hrow_d.rearrange("b l d -> (b l) d")
        wn8_sb = []
        for b in range(BPC):
            t8v = sm.tile([1, 8], bf16, tag="t8v", name="t8v")
            t8p = sm.tile([1, 8], u32, tag="t8p", name="t8p")
            nc.vector.max(t8v[:], fL_sb[b][:])
            nc.vector.max_index(t8p[:], t8v[:], fL_sb[b][:])
            t8pf = sm.tile([1, 8], f32, tag="t8pf", name="t8pf")
            nc.vector.tensor_copy(t8pf[:], t8p[:])
            nc.vector.tensor_scalar_add(t8pf[:], t8pf[:], float(b * L))
            t8pi = sm.tile([1, 8], u32, tag="t8pi", name="t8pi")
            nc.vector.tensor_copy(t8pi[:], t8pf[:])
            idx8 = sm.tile([NCAND, 1], u32, tag="idx8", name="idx8")
            nc.sync.dma_start(idx8[:, 0:1], t8pi[0:1, :])
            rows = sm.tile([NCAND, D], f32, tag="rows", name="rows")
            nc.gpsimd.indirect_dma_start(
                out=rows[:], out_offset=None, in_=hrow_flat,
                in_offset=bass.IndirectOffsetOnAxis(ap=idx8[:, 0:1], axis=0))
            # exact fp32 logits for the 8 candidates (f32 products so the
            # reduce is fp32-exact; tensor_tensor_reduce is avoided — it
            # crashes the device on this runtime)
            prod = sm.tile([NCAND, D], f32, tag="ek", name="prod")
            e8 = sm.tile([NCAND, 1], f32, tag="e8", name="e8")
            nc.vector.tensor_mul(prod[:], rows[:], wsf8[:])
            nc.vector.tensor_reduce(e8[:], prod[:], AX.X, ALU.add)
            e8r = sm.tile([1, 8], f32, tag="e8r", name="e8r")
            nc.sync.dma_start(e8r[0:1, :], e8[:])
            s8 = sm.tile([1, 8], f32, tag="s8", name="s8")
            nc.vector.max(s8[:], e8r[:])
            thr = sm.tile([1, 1], f32, tag="thr", name="thr")
            nc.vector.tensor_add(thr[:], s8[0:1, K - 1:K], s8[0:1, K:K + 1])
            nc.vector.tensor_scalar_mul(thr[:], thr[:], 0.5)
            msk = sm.tile([1, 8], f32, tag="msk", name="msk")
            nc.vector.tensor_scalar(msk[:], e8r[:], thr[:], None, ALU.is_gt)
            negmx = sm.tile([1, 1], f32, tag="negmx", name="negmx")
            nc.vector.tensor_scalar_mul(negmx[:], s8[0:1, 0:1], -1.0)
            ew = sm.tile([1, 8], f32, tag="ew", name="ew")
            nc.scalar.activation(ew[:], e8r[:], AF.Exp, bias=negmx[:], scale=1.0)
            w8m = sm.tile([1, 8], f32, tag="w8m", name="w8m")
            nc.vector.tensor_mul(w8m[:], ew[:], msk[:])
            sw = sm.tile([1, 1], f32, tag="sw", name="sw")
            nc.vector.tensor_reduce(sw[:], w8m[:], AX.X, ALU.add)
            rsw = sm.tile([1, 1], f32, tag="rsw", name="rsw")
            nc.vector.reciprocal(rsw[:], sw[:])
            wn = sm.tile([1, 8], f32, tag="wn", name="wn")
            nc.vector.tensor_scalar_mul(wn[:], w8m[:], rsw[:])
            wn8 = sm.tile([NCAND, 1], f32, tag=f"wn8_{b}", name=f"wn8_{b}")
            wn8_sb.append(wn8)
            nc.sync.dma_start(wn8[:, 0:1], wn[0:1, :])

            # S3 for this example: PE transpose of the gathered fp32 rows
            psr = psm.tile([128, DC, NCAND], f32, tag="sm", name="psr")
            for dc in range(DC):
                nc.tensor.matmul(psr[:, dc, :],
                                 rows[:, dc * 128:(dc + 1) * 128], i8[:],
                                 is_transpose=True, start=True, stop=True,
                                 skip_group_check=True)
            nc.vector.tensor_copy(srhs[:, :, b, 0, :], psr[:])
            nc.vector.tensor_copy(sr_hf[:], srhs[:, :, b, 0, :])
            nc.vector.tensor_sub(sr_lf[:], psr[:], sr_hf[:])
            nc.vector.tensor_copy(srhs[:, :, b, 1, :], sr_lf[:])

        # ---- S4: Q^T then P^T chains (weights streamed, both examples)
        def wchain(w_d, rhs, tag):
            ps4 = psm.tile([128, DC, BPC, 2, NCAND], f32, tag="sm", name="ps4")
            for dci in range(DC):
                wt = wstg.tile([128, D], bf16, tag="wt", name="wt")
                nc.gpsimd.dma_start(wt[:], w_d[dci * 128:(dci + 1) * 128, :])
                for dco in range(DC):
                    # one global start per psum tile: a later start=True would
                    # clobber sibling regions' accumulation state in the bank
                    nc.tensor.matmul(ps4[:, dco, :, :, :],
                                     wt[:, dco * 128:(dco + 1) * 128],
                                     rhs[:, dci, :, :, :],
                                     start=(dci == 0 and dco == 0),
                                     stop=(dci == DC - 1),
                                     skip_group_check=True)
            qf = sm.tile([128, DC, BPC, NCAND], f32, tag=tag + "f", name=tag + "f")
            nc.vector.tensor_copy(qf[:], ps4[:, :, :, 0, :])
            nc.vector.tensor_add(qf[:], qf[:], ps4[:, :, :, 1, :])
            pair = sm.tile([128, DC, BPC, 2, NCAND], bf16, tag=tag, name=tag)
            nc.vector.tensor_copy(pair[:, :, :, 0, :], qf[:])
            hf = sm.tile([128, DC, BPC, NCAND], f32, tag=tag + "h", name=tag + "h")
            nc.vector.tensor_copy(hf[:], pair[:, :, :, 0, :])
            nc.vector.tensor_sub(qf[:], qf[:], hf[:])
            nc.vector.tensor_copy(pair[:, :, :, 1, :], qf[:])
            return pair

        qpair = wchain(wq_d, srhs, "qp")
        ppair = wchain(wkt_d, qpair, "pp")

        # ---- S5 both examples first (PE back-to-back), chunk maxes on the fly
        sct_sb, mxc_sb = [], []
        for b in range(BPC):
            sct = sm.tile([NCAND, L], bf16, tag=f"sct{b}", name=f"sct{b}")
            sct_sb.append(sct)
            mxc = sm.tile([NCAND, NJ], f32, tag=f"mxc{b}", name=f"mxc{b}")
            mxc_sb.append(mxc)
            for j in range(NJ):
                ps5 = pbig.tile([16, 512], f32, tag="mm", name="ps5")
                for dc in range(DC):
                    nc.tensor.matmul(ps5[:], ppair[:, dc, b, :, :],
                                     ht_sb[b][:, dc, j * 512:(j + 1) * 512],
                                     start=(dc == 0), stop=(dc == DC - 1))
                # fold hi+lo rows: engines cannot cross partition bases, so
                # stage via scalar copy + DMA partition move, then DVE add
                cp5 = sm.tile([16, 512], f32, tag="cp5", bufs=2, name="cp5")
                nc.scalar.copy(cp5[:], ps5[:])
                cp5b = sm.tile([NCAND, 512], f32, tag="cp5b", bufs=2, name="cp5b")
                nc.sync.dma_start(cp5b[:], cp5[NCAND:16, :])
                nc.vector.tensor_add(sct[:, j * 512:(j + 1) * 512],
                                     cp5[0:NCAND, :], cp5b[:])
                nc.vector.tensor_reduce(mxc[:, j:j + 1],
                                        sct[:, j * 512:(j + 1) * 512],
                                        AX.X, ALU.max)

        # ---- softmax + m broadcast + S6 per example (shared big scratch)
        amix = sm.tile([128, DC, BPC], f32, tag="amix", name="amix")
        for b in range(BPC):
            mx8 = sm.tile([NCAND, 1], f32, tag="mx8", name="mx8")
            nc.vector.tensor_reduce(mx8[:], mxc_sb[b][:], AX.X, ALU.max)
            nbias = sm.tile([NCAND, 1], f32, tag="nbias", name="nbias")
            nc.vector.tensor_scalar_mul(nbias[:], mx8[:], -SCALE)
            ek = sm.tile([NCAND, L], bf16, tag="ek", name="ek")
            z8 = sm.tile([NCAND, 1], f32, tag="z8", name="z8")
            nc.scalar.activation(ek[:], sct_sb[b][:], AF.Exp, bias=nbias[:],
                                 scale=SCALE, accum_out=z8[:])
            rz8 = sm.tile([NCAND, 1], f32, tag="rz8", name="rz8")
            nc.vector.reciprocal(rz8[:], z8[:])
            c8 = sm.tile([NCAND, 1], f32, tag="c8", name="c8")
            nc.vector.tensor_mul(c8[:], wn8_sb[b][:], rz8[:])
            nc.vector.tensor_scalar_mul(ek[:], ek[:], c8[:])   # ek *= c8
            # m as [32, 128] (nat-layout rows), then transpose to [128, 32]
            mt32 = sm.tile([32, 128], bf16, tag="mt32", name="mt32")
            for j in range(NJ):
                pm = pbig.tile([16, 512], f32, tag="mm", name="pm")
                nc.tensor.matmul(pm[0:1, :], ones8[:],
                                 ek[:, j * 512:(j + 1) * 512],
                                 start=True, stop=True)
                m1 = sm.tile([1, 512], bf16, tag="m1", bufs=2, name="m1")
                nc.scalar.copy(m1[:], pm[0:1, :])
                nc.sync.dma_start(mt32[4 * j:4 * j + 4, :], m1[:])
            pt = psm.tile([128, 32], bf16, tag="sm", name="pt")
            nc.tensor.matmul(pt[:], mt32[:], i32[:], is_transpose=True,
                             start=True, stop=True)
            mt = sm.tile([128, 32], bf16, tag="mt", name="mt")
            nc.vector.tensor_copy(mt[:], pt[:])
            # S6: a_mix = sum_l H[l, d] * m[l] on PE, streaming natural H
            ps6 = psm.tile([128, DC, 1], f32, tag="sm", name="ps6")
            NLC = L // 128
            for lc2 in range(NLC // 2):
                nat = natp.tile([128, 2, D], bf16, tag="nat", name="nat")
                nc.gpsimd.dma_start(
                    nat[:],
                    nat_d[b, lc2 * 256:(lc2 + 1) * 256, :]
                    .rearrange("(i p) d -> p i d", p=128))
                for i in range(2):
                    lc = lc2 * 2 + i
                    for dc in range(DC):
                        nc.tensor.matmul(ps6[:, dc, :],
                                         nat[:, i, dc * 128:(dc + 1) * 128],
                                         mt[:, lc:lc + 1],
                                         start=(lc == 0 and dc == 0),
                                         stop=(lc == NLC - 1),
                                         skip_group_check=True)
            nc.vector.tensor_copy(amix[:, :, b:b + 1], ps6[:])

        # ---- split helper [128, DC, BPC] f32 -> [128, 2, DC, BPC] bf16
        def split2(src, tag):
            pair = sm.tile([128, DC, 2, BPC], bf16, tag=tag, name=tag)
            nc.vector.tensor_copy(pair[:, :, 0, :], src[:])
            hf = sm.tile([128, DC, BPC], f32, tag=tag + "h", name=tag + "h")
            nc.vector.tensor_copy(hf[:], pair[:, :, 0, :])
            nc.vector.tensor_sub(hf[:], src[:], hf[:])
            nc.vector.tensor_copy(pair[:, :, 1, :], hf[:])
            return pair

        arhs = split2(amix, "arhs")

        # ---- S7: c_mix (wv), g (wct), shared weight streams for both examples
        def wchain2(w_d, rhs, tag):
            ps7 = psm.tile([128, DC, 2, BPC], f32, tag="sm", name="ps7")
            for dci in range(DC):
                wt = wstg.tile([128, D], bf16, tag="wt", name="wt")
                nc.gpsimd.dma_start(wt[:], w_d[dci * 128:(dci + 1) * 128, :])
                for dco in range(DC):
                    nc.tensor.matmul(ps7[:, dco, :, :],
                                     wt[:, dco * 128:(dco + 1) * 128],
                                     rhs[:, dci, :, :],
                                     start=(dci == 0 and dco == 0),
                                     stop=(dci == DC - 1),
                                     skip_group_check=True)
            outf = sm.tile([128, DC, BPC], f32, tag=tag, name=tag)
            nc.vector.tensor_copy(outf[:], ps7[:, :, 0, :])
            nc.vector.tensor_add(outf[:], outf[:], ps7[:, :, 1, :])
            return outf

        cmix = wchain2(wv_d, arhs, "cm")
        crhs = split2(cmix, "crhs")
        g_f = wchain2(wct_d, crhs, "gg")
        gs = sm.tile([128, DC, BPC], f32, tag="gs", name="gs")
        nc.vector.tensor_scalar_mul(gs[:], g_f[:], SCALE)
        grhs = split2(gs, "grhs")

        # ---- S8: end logits from resident ht
        for b in range(BPC):
            for j in range(NJ):
                ps8 = pbig.tile([16, 512], f32, tag="mm", name="ps8")
                for dc in range(DC):
                    nc.tensor.matmul(ps8[0:2, :], grhs[:, dc, :, b],
                                     ht_sb[b][:, dc, j * 512:(j + 1) * 512],
                                     start=(dc == 0), stop=(dc == DC - 1))
                cp8 = sm.tile([2, 512], f32, tag="cp5", bufs=2, name="cp8")
                nc.scalar.copy(cp8[:], ps8[0:2, :])
                cp8b = sm.tile([1, 512], f32, tag="cp5b", bufs=2, name="cp8b")
                nc.sync.dma_start(cp8b[:], cp8[1:2, :])
                etmp = sm.tile([1, 512], f32, tag="ltmp", bufs=2, name="etmp")
                nc.vector.tensor_add(etmp[:], cp8[0:1, :], cp8b[:])
                nc.sync.dma_start(el_d[b:b + 1, j * 512:(j + 1) * 512], etmp[:])

    if os.environ.get("KERNEL_BUILD_INFO"):
        print(f"[kernel] sbuf remaining: {nc.sbuf_bytes_remaining} bytes")
    nc.compile()
    _NC_CACHE["nc"] = nc
    return nc


def _np_reference(H, attention_mask, w_start, b_start, w_q, b_q, w_k, b_k,
                  w_v, b_v, w_cmp, b_cmp):
    NEG = -1e9
    H = H.astype(np.float32)
    pad = attention_mask == 0
    sl = (H @ w_start + b_start)[..., 0]
    sl = np.where(pad, NEG, sl)
    x = sl - sl.max(-1, keepdims=True)
    e = np.exp(x); sp = e / e.sum(-1, keepdims=True)
    idx = np.argsort(-sp, axis=-1, kind="stable")[:, :K]
    tp = np.take_along_axis(sp, idx, axis=1)
    sr = np.take_along_axis(H, idx[..., None], axis=1)
    Q = sr @ w_q + b_q
    K_ = H @ w_k + b_k
    V = H @ w_v + b_v
    sc = np.einsum('bkd,bld->bkl', Q, K_) * SCALE
    sc = np.where(pad[:, None, :], NEG, sc)
    sc = sc - sc.max(-1, keepdims=True)
    a = np.exp(sc); a = a / a.sum(-1, keepdims=True)
    ctx_ = np.einsum('bkl,bld->bkd', a, V)
    tcmp = H @ w_cmp + b_cmp
    es = np.einsum('bkd,bld->bkl', ctx_, tcmp) * SCALE
    es = np.where(pad[:, None, :], NEG, es)
    w = tp / (tp.sum(-1, keepdims=True) + 1e-9)
    el = np.einsum('bk,bkl->bl', w, es)
    el = np.where(pad, NEG, el)
    return sl, el


def kernel(**inputs):
    H = np.asarray(inputs["H"], np.float32)
    mask = np.asarray(inputs["attention_mask"])
    b_start = np.asarray(inputs["b_start"], np.float32)
    biases_zero = all(np.all(np.asarray(inputs[n]) == 0)
                      for n in ["b_q", "b_k", "b_v", "b_cmp"])
    if not bool((mask == 1).all()) or not biases_zero:
        sl, el = _np_reference(**{k: np.asarray(v) for k, v in inputs.items()})
        return np.asarray(sl, np.float32), np.asarray(el, np.float32)

    w_start = np.asarray(inputs["w_start"], np.float32)
    w_q = np.asarray(inputs["w_q"], np.float32)
    w_k = np.asarray(inputs["w_k"], np.float32)
    w_v = np.asarray(inputs["w_v"], np.float32)
    w_cmp = np.asarray(inputs["w_cmp"], np.float32)

    hi = H.astype(bfnp)
    ht = np.ascontiguousarray(hi.transpose(0, 2, 1)).reshape(B, DC, 128, L)
    wsb = w_start[:, 0].astype(bfnp).reshape(DC, 128, 1)
    wsf8 = np.ascontiguousarray(
        np.broadcast_to(w_start[:, 0], (NCAND, D))).astype(np.float32)

    nc = _build_nc()
    in_maps = []
    for c in range(NCORES):
        s = slice(c * BPC, (c + 1) * BPC)
        in_maps.append({
            "ht": ht[s], "hrow": H[s], "nat": hi[s],
            "wq": w_q.astype(bfnp),
            "wkt": np.ascontiguousarray(w_k.T).astype(bfnp),
            "wv": w_v.astype(bfnp),
            "wct": np.ascontiguousarray(w_cmp.T).astype(bfnp),
            "wsb": wsb, "wsf8": wsf8,
            "i8": np.eye(NCAND, dtype=np.float32),
            "ones8": np.ones((NCAND, 1), bfnp),
            "i32": np.eye(32, dtype=np.float32).astype(bfnp),
        })
    import time as _time
    _t0 = _time.time()
    kw = {}
    if os.environ.get("KERNEL_PROFILE"):
        kw = dict(trace=True,
                  tmpdir=os.environ.get("KERNEL_PROFILE_DIR") or None,
                  trace_cores=[int(x) for x in
                               os.environ.get("KERNEL_TRACE_CORES", "0").split(",")])
    res = run_bass_kernel_spmd(nc, in_maps, core_ids=list(range(NCORES)), **kw)
    LAST["res"] = res
    if os.environ.get("KERNEL_TIME"):
        print(f"[kernel] device dispatch+exec wall: {_time.time() - _t0:.3f}s")
    sl = np.concatenate([r["sl"] for r in res.results], 0) + b_start[0]
    el = np.concatenate([r["el"] for r in res.results], 0)
    return sl.astype(np.float32), el.astype(np.float32)
